# revision 1
# baseline (speedup 1.0000x reference)
"""AERO-GNN forward pass on 8 TRN2 NeuronCores (Bass/Tile).

Sharding: edges partitioned by target-node range; core r owns target nodes
[r*NPC, (r+1)*NPC) and all edges pointing at them, so deg/h_new scatters are
core-local. Per hop, the two row-indexed tables (z_scale and dinv*h) are
replicated as bf16 via AllGather; row gathers use SWDGE dma_gather on 4
queues; segment sums (deg, h_new) run on the TensorEngine as per-tile
one-hot matmuls accumulated in PSUM node-blocks. All cores execute one SPMD
instruction stream; per-core structure differences are absorbed by padding
tile counts to cross-core maxima (padded tiles carry all-zero one-hots).
"""
import os
import sys

sys.path.insert(0, "/opt/trn_rl_repo")

import ml_dtypes
import numpy as np

import concourse.bacc as bacc
import concourse.bass as bass
import concourse.mybir as mybir
import concourse.tile as tile
from concourse import library_config

F32 = mybir.dt.float32
BF16 = mybir.dt.bfloat16
I16 = mybir.dt.int16
AX = mybir.AxisListType
OP = mybir.AluOpType
AF = mybir.ActivationFunctionType

# Problem constants (hardcoded per harness contract).
N, E = 50000, 800000
H, C = 8, 16
KHOPS = 4
DIN, DH, DOUT = 256, 128, 40
LAMBD = 1.0
NCORES = 8

IDX_LIMIT = 32768   # int16 gather index limit -> lo/hi row split
BLK = 128           # target-node block width (PSUM partitions)
BLK_PER_CHUNK = 2   # node blocks per gather chunk


def _decay(k):
    return float(np.log(LAMBD / (k + 1) + (1 + 1e-06)))


def _mid_bcast(ap, count, pos):
    """Insert a step-0 (broadcast) dim of `count` at position `pos`."""
    new_ap = [list(d) for d in ap.ap]
    new_ap = new_ap[:pos] + [[0, count]] + new_ap[pos:]
    return bass.AP(ap.tensor, ap.offset, new_ap)


def _wrap_idx(idx_flat):
    """[n] (n%16==0) -> [128, n/16] int16; slot i -> (part i%16, col i//16),
    replicated into all 8 Q7 groups."""
    n = idx_flat.shape[0]
    w = idx_flat.reshape(n // 16, 16).T.astype(np.int16)
    return np.tile(w, (8, 1))



def _patch_tile_swdge_sems():
    """Make Tile's DMASW semaphore lanes queue-aware: SWDGE queue q owns
    sems {2q, 2q+1}. Without this, round-robin assignment hands one sem to
    instructions on different SWDGE queues, which the HW/sim reject."""
    import concourse.tile_sem_assignment as tsa
    if getattr(tsa.TileClockTick, "_swdge_qpatched", False):
        return
    orig = tsa.TileClockTick._assign_tick

    def patched(self, inst):
        try:
            is_pool_dma = (isinstance(inst, tsa.DMAInst)
                           and inst.engine == mybir.EngineType.Pool)
        except Exception:
            is_pool_dma = False
        if is_pool_dma:
            q = int(getattr(inst, "queue_num", 0) or 0) % 4
            tog = self.__dict__.setdefault("_qtog", {})
            t = tog.get(q, 0)
            tog[q] = t ^ 1
            self.next_sw_dma_idx = 2 * q + t
        return orig(self, inst)

    tsa.TileClockTick._assign_tick = patched
    tsa.TileClockTick._swdge_qpatched = True


# ---------------------------------------------------------------------------
# Host-side static preprocessing
# ---------------------------------------------------------------------------

def _preprocess(edge_index, npc, n_nodes):
    """Per-core edge structure: col-sorted edges grouped into node blocks,
    split lo/hi by row id, cut into 128-edge tiles. Returns per-core dicts
    with per-(chunk, block, half) tile groups."""
    row = np.asarray(edge_index[0], dtype=np.int64)
    col = np.asarray(edge_index[1], dtype=np.int64)
    loops = np.arange(n_nodes, dtype=np.int64)
    row = np.concatenate([row, loops])
    col = np.concatenate([col, loops])

    nblk = (npc + BLK - 1) // BLK
    nchunk = (nblk + BLK_PER_CHUNK - 1) // BLK_PER_CHUNK
    cores = []
    for r in range(NCORES):
        lo_n = r * npc
        sel = (col >= lo_n) & (col < lo_n + npc)
        er = row[sel]
        ec = col[sel] - lo_n
        order = np.argsort(ec, kind="stable")
        er, ec = er[order], ec[order]
        groups = {}  # (blk, half) -> list of (rows[128], colloc[128], mask[128])
        for b in range(nblk):
            for half in (0, 1):
                m = (ec // BLK == b)
                m &= (er < IDX_LIMIT) if half == 0 else (er >= IDX_LIMIT)
                rr = er[m]
                cc = ec[m] - b * BLK
                tiles = []
                for i in range(0, max(len(rr), 1), 128):
                    pr = np.zeros(128, np.int64)
                    pc = np.zeros(128, np.int64)
                    pm = np.zeros(128, np.float32)
                    n_e = min(128, len(rr) - i)
                    if n_e > 0:
                        pr[:n_e] = rr[i:i + n_e]
                        pc[:n_e] = cc[i:i + n_e]
                        pm[:n_e] = 1.0
                    tiles.append((pr, pc, pm))
                    if len(rr) == 0:
                        break
                if len(rr) == 0:
                    tiles = []
                groups[(b, half)] = tiles
        cores.append(dict(groups=groups, nblk=nblk, nchunk=nchunk))
    return cores


def _unify(cores):
    """Pad tile counts to cross-core maxima so all cores share one layout.

    Returns (layout, per_core) where layout drives the instruction stream and
    per_core holds the data arrays (pstat, row/col idx)."""
    nblk = cores[0]["nblk"]
    nchunk = cores[0]["nchunk"]
    ntile = {}  # (blk, half) -> padded count
    for b in range(nblk):
        for half in (0, 1):
            ntile[(b, half)] = max(len(c["groups"][(b, half)]) for c in cores)

    # chunk layout: for chunk ci covering blocks [b0, b1):
    #   slots = [lo tiles of b0..b1-1] ++ [hi tiles of b0..b1-1]
    chunks = []   # (t0, S, n_lo_tiles, blocks, tile_slot[(b,half)] -> slot0)
    t_base = 0
    for ci in range(nchunk):
        b0 = ci * BLK_PER_CHUNK
        b1 = min(b0 + BLK_PER_CHUNK, nblk)
        slotmap = {}
        s = 0
        for half in (0, 1):
            for b in range(b0, b1):
                slotmap[(b, half)] = s
                s += ntile[(b, half)]
        n_lo = sum(ntile[(b, 0)] for b in range(b0, b1))
        chunks.append(dict(t0=t_base, S=s, nlo=n_lo, blocks=list(range(b0, b1)),
                           slot=slotmap))
        t_base += s
    T = t_base

    layout = dict(nblk=nblk, nchunk=nchunk, T=T, chunks=chunks, ntile=ntile)

    per_core = []
    for c in cores:
        pstat = np.zeros((T, 128, BLK), np.float32)
        rows = np.zeros((T, 128), np.int64)
        colg = np.zeros((T, 128), np.int64)
        for ch in chunks:
            for (b, half), s0 in ch["slot"].items():
                tiles = c["groups"][(b, half)]
                for t, (pr, pc, pm) in enumerate(tiles):
                    tt = ch["t0"] + s0 + t
                    rows[tt] = pr
                    colg[tt] = pc + b * BLK
                    pstat[tt, np.arange(128), pc] = pm
        # row idx arrays: per chunk [lo slots][hi slots]
        rowi_parts, coli_parts = [], []
        chunk_idx_meta = []
        rcw = ccw = 0
        for ch in chunks:
            t0, S, nlo = ch["t0"], ch["S"], ch["nlo"]
            r_lo = rows[t0:t0 + nlo].reshape(-1)
            r_hi = np.maximum(rows[t0 + nlo:t0 + S].reshape(-1) - IDX_LIMIT, 0)
            cg = colg[t0:t0 + S].reshape(-1)
            lo_w = _wrap_idx(r_lo) if nlo else np.zeros((128, 0), np.int16)
            hi_w = (_wrap_idx(r_hi) if (S - nlo) else np.zeros((128, 0), np.int16))
            c_w = _wrap_idx(cg)
            chunk_idx_meta.append((rcw, lo_w.shape[1], hi_w.shape[1],
                                   ccw, c_w.shape[1]))
            rcw += lo_w.shape[1] + hi_w.shape[1]
            ccw += c_w.shape[1]
            rowi_parts += [lo_w, hi_w]
            coli_parts += [c_w]
        per_core.append(dict(
            pstat=np.ascontiguousarray(
                pstat.transpose(1, 0, 2).reshape(128, T * BLK)
            ).astype(ml_dtypes.bfloat16),
            row_idx=np.concatenate(rowi_parts, axis=1).astype(np.int16),
            col_idx=np.concatenate(coli_parts, axis=1).astype(np.int16),
        ))
    layout["chunk_idx"] = chunk_idx_meta
    layout["row_idx_w"] = per_core[0]["row_idx"].shape[1]
    layout["col_idx_w"] = per_core[0]["col_idx"].shape[1]
    return layout, per_core


# ---------------------------------------------------------------------------
# Device graph (SPMD; one instruction stream for all 8 cores)
# ---------------------------------------------------------------------------

def _build(lay, npc, n_nodes):
    nblk = lay["nblk"]
    T = lay["T"]
    LIM = min(IDX_LIMIT, n_nodes)

    _patch_tile_swdge_sems()
    nc = bacc.Bacc("TRN2", target_bir_lowering=False, debug=False,
                   num_swdge_queues=4)

    dram_in = lambda name, shape, dt: nc.dram_tensor(name, shape, dt,
                                                     kind="ExternalInput")
    xT = dram_in("xT", [DIN, npc], F32)
    W0 = dram_in("W0", [DIN, DH], F32)
    W1 = dram_in("W1", [DH, DH], F32)
    Wout = dram_in("Wout", [DH, DOUT], F32)
    b0_col = dram_in("b0_col", [DH, 1], F32)
    b1_col = dram_in("b1_col", [DH, 1], F32)
    ident_in = dram_in("ident", [128, 128], F32)
    att0_rep = dram_in("att0_rep", [128, DH], F32)
    attsk_rep = dram_in("attsk_rep", [128, KHOPS * DH], BF16)
    hatt_h_rep = dram_in("hatt_h_rep", [128, KHOPS * DH], F32)
    hatt_z_rep = dram_in("hatt_z_rep", [128, KHOPS * DH], F32)
    hbias_rep = dram_in("hbias_rep", [128, (KHOPS + 1) * H], F32)
    pstat_in = dram_in("pstat", [128, T * BLK], BF16)
    row_idx_in = dram_in("row_idx", [128, max(lay["row_idx_w"], 1)], I16)
    col_idx_in = dram_in("col_idx", [128, max(lay["col_idx_w"], 1)], I16)

    out_ext = nc.dram_tensor("out", [DOUT, npc], F32, kind="ExternalOutput")
    hdbg = nc.dram_tensor("hdbg", [npc, DH], F32, kind="ExternalOutput") if os.environ.get("KERNEL_DEBUG") else None

    npc_pad = nblk * BLK
    h_hbm = nc.dram_tensor("h_hbm", [npc_pad, DH], F32)
    z_hbm = nc.dram_tensor("z_hbm", [npc_pad, DH], F32)
    zs_own = nc.dram_tensor("zs_own", [npc_pad, DH], BF16)
    ht_own = nc.dram_tensor("ht_own", [npc_pad, DH], BF16)
    zs_full = nc.dram_tensor("zs_full", [n_nodes, DH], BF16, addr_space="Shared")
    a_hbm = nc.dram_tensor("a_hbm", [128, max(T * H, 1)], BF16)
    ht_full = nc.dram_tensor("ht_full", [n_nodes, DH], BF16, addr_space="Shared")

    with tile.TileContext(nc) as tc:
        with (
            tc.tile_pool(name="const", bufs=1) as constp,
            tc.tile_pool(name="state", bufs=2) as statep,
            tc.tile_pool(name="gath", bufs=2) as gathp,
            tc.tile_pool(name="work", bufs=2) as workp,
            tc.tile_pool(name="small", bufs=2) as smallp,
            tc.tile_pool(name="hold", bufs=1) as holdp,
            tc.tile_pool(name="psA", bufs=2, space="PSUM") as psA,
            tc.tile_pool(name="psB", bufs=2, space="PSUM") as psB,
        ):
            gp, ve, se, te = nc.gpsimd, nc.vector, nc.scalar, nc.tensor
            sy = nc.sync

            gp.load_library(library_config.mlp)

            def ctile(shape, dt, tag, src):
                t = constp.tile(shape, dt, tag=tag)
                sy.dma_start(t[:], src[:])
                return t

            w0_sb = constp.tile([128, 2, DH], F32, tag="w0")
            for kc in range(2):
                sy.dma_start(w0_sb[:, kc, :], W0[kc * 128:(kc + 1) * 128, :])
            w1_sb = ctile([DH, DH], F32, "w1", W1)
            wout_sb = ctile([DH, DOUT], F32, "wout", Wout)
            rowi_sb = ctile([128, max(lay["row_idx_w"], 1)], I16, "rowi",
                            row_idx_in)
            coli_sb = ctile([128, max(lay["col_idx_w"], 1)], I16, "coli",
                            col_idx_in)
            b0_sb = ctile([DH, 1], F32, "b0", b0_col)
            b1_sb = ctile([DH, 1], F32, "b1", b1_col)
            ident_sb = ctile([128, 128], F32, "ident", ident_in)
            att0_sb = ctile([128, DH], F32, "att0", att0_rep)
            attsk_sb = ctile([128, KHOPS * DH], BF16, "attsk", attsk_rep)
            hatth_sb = ctile([128, KHOPS * DH], F32, "hatth", hatt_h_rep)
            hattz_sb = ctile([128, KHOPS * DH], F32, "hattz", hatt_z_rep)
            hbias_sb = ctile([128, (KHOPS + 1) * H], F32, "hbias", hbias_rep)

            def elu_(dst, src, pool, tag):
                # elu(x) = (max(x,0) - 1) + min(e^x, 1); inputs are O(1) so
                # the direct Exp cannot overflow. 1 ACT + 2 DVE passes.
                p = src.shape[0]
                rest = list(src.shape[1:])
                mn = pool.tile([128] + rest, F32, tag=tag + "_mn")
                ex = pool.tile([128] + rest, F32, tag=tag + "_ex")
                se.activation(ex[:p], src, AF.Exp)
                ve.tensor_scalar(mn[:p], src, 0.0, -1.0, OP.max, OP.add)
                ve.scalar_tensor_tensor(dst, ex[:p], 1.0, mn[:p], OP.min, OP.add)

            def hc(apv):
                return apv.rearrange("p (h c) -> p h c", c=C)

            # =========== MLP + k=0 ===========
            for b in range(nblk):
                nb = min(BLK, npc - b * BLK)
                xt_sb = statep.tile([128, 2, BLK], F32, tag="xt")
                for kc in range(2):
                    sy.dma_start(xt_sb[:, kc, :nb],
                                 xT[kc * 128:(kc + 1) * 128, b * BLK:b * BLK + nb])
                ps = psA.tile([128, BLK], F32, tag="mm")
                for kc in range(2):
                    te.matmul(ps[:, :nb], w0_sb[:, kc, :], xt_sb[:, kc, :nb],
                              start=(kc == 0), stop=(kc == 1))
                h1t = statep.tile([128, BLK], F32, tag="h1t")
                ve.tensor_tensor(h1t[:, :nb], ps[:, :nb],
                                 b0_sb[:, 0:1].broadcast_to([DH, nb]), OP.add)
                elu_(h1t[:, :nb], h1t[:, :nb], statep, "melu")
                ps2 = psA.tile([128, BLK], F32, tag="mm")
                te.matmul(ps2[:, :nb], w1_sb[:], h1t[:, :nb], start=True, stop=True)
                h2t = statep.tile([128, BLK], F32, tag="h2t")
                ve.tensor_tensor(h2t[:, :nb], ps2[:, :nb],
                                 b1_sb[:, 0:1].broadcast_to([DH, nb]), OP.add)
                ps3 = psA.tile([128, BLK], F32, tag="mm")
                te.matmul(ps3[:, :], h2t[:, :], ident_sb[:, :],
                          is_transpose=True, start=True, stop=True)
                hfin = statep.tile([128, DH], F32, tag="hfin")
                ve.tensor_copy(hfin[:, :], ps3[:, :])
                sy.dma_start(h_hbm[b * BLK:(b + 1) * BLK, :], hfin[:, :])
                if hdbg is not None:
                    sy.dma_start(hdbg[b * BLK:b * BLK + nb, :], hfin[:nb])

            # k=0 gate/update, grouped over node blocks
            GB = 8
            for g0i in range(0, nblk, GB):
                gn = min(GB, nblk - g0i)

                def grp0(t):
                    return t[:, :gn, :]

                def dram_grp0(dt):
                    return (dt[g0i * BLK:(g0i + gn) * BLK, :]
                            .rearrange("(g p) d -> p g d", p=BLK))

                def hcg0(apv):
                    return apv[:, :gn, :].rearrange("p g (h c) -> p g h c", c=C)

                hng = statep.tile([128, GB, DH], F32, tag="ht_h")
                sy.dma_start(grp0(hng), dram_grp0(h_hbm))
                eh = statep.tile([128, GB, DH], F32, tag="tl_e1")
                elu_(grp0(eh), grp0(hng), statep, "tl1")
                ve.tensor_tensor(grp0(eh), grp0(eh),
                                 _mid_bcast(att0_sb[:, :], gn, 1), OP.mult)
                g0v = statep.tile([128, GB, H], F32, tag="tl_gv")
                ve.tensor_reduce(g0v[:, :gn, :], hcg0(eh), AX.X, OP.add)
                ve.tensor_tensor(g0v[:, :gn, :], g0v[:, :gn, :],
                                 _mid_bcast(hbias_sb[:, 0:H], gn, 1), OP.add)
                ztg = statep.tile([128, GB, DH], F32, tag="tl_zb")
                ve.tensor_tensor(hcg0(ztg), hcg0(hng),
                                 g0v[:, :gn, :].broadcast_to([128, gn, H, C]),
                                 OP.mult)
                sy.dma_start(dram_grp0(z_hbm), grp0(ztg))
                zsg = statep.tile([128, GB, DH], BF16, tag="tl_zso")
                ve.tensor_scalar(grp0(zsg), grp0(ztg), _decay(0), None, OP.mult)
                sy.dma_start(dram_grp0(zs_own), grp0(zsg))

            # =========== hops ===========
            for k in range(1, KHOPS + 1):
                gp.collective_compute(
                    "AllGather", OP.bypass,
                    replica_groups=[list(range(NCORES))],
                    ins=[zs_own[0:npc, :]], outs=[zs_full[:]],
                )

                deg_sb = holdp.tile([128, nblk, H], F32, tag="deg")
                for ci, ch in enumerate(lay["chunks"]):
                    t0, S, nlo = ch["t0"], ch["S"], ch["nlo"]
                    rc0, loW, hiW, cc0, cW = lay["chunk_idx"][ci]
                    zr = gathp.tile([128, S, DH], BF16, tag="g_a")
                    zc = gathp.tile([128, S, DH], BF16, tag="g_b")
                    q = 3 * ci
                    if loW:
                        gp.dma_gather(zr[:, :nlo, :], zs_full[0:LIM, :],
                                      rowi_sb[:, rc0:rc0 + loW], loW * 16, loW * 16,
                                      DH, single_packet=False, queue_num=q % 4)
                    if hiW:
                        gp.dma_gather(zr[:, nlo:S, :],
                                      zs_full[LIM:n_nodes, :],
                                      rowi_sb[:, rc0 + loW:rc0 + loW + hiW],
                                      hiW * 16, hiW * 16, DH,
                                      single_packet=False, queue_num=(q + 1) % 4)
                    gp.dma_gather(zc[:, :, :], zs_own[:, :],
                                  coli_sb[:, cc0:cc0 + cW], cW * 16, cW * 16,
                                  DH, single_packet=False, queue_num=(q + 2) % 4)
                    ve.tensor_tensor(zr[:], zr[:], zc[:], OP.add)
                    tmp = workp.tile([128, S, DH], BF16, tag="welu")
                    se.activation(tmp[:], zr[:], AF.Exp)
                    ve.tensor_scalar(zr[:], zr[:], 0.0, -1.0, OP.max, OP.add)
                    ve.scalar_tensor_tensor(zr[:], tmp[:], 1.0, zr[:],
                                            OP.min, OP.add)
                    ve.tensor_tensor(
                        zr[:], zr[:],
                        _mid_bcast(attsk_sb[:, (k - 1) * DH:k * DH], S, 1),
                        OP.mult)
                    araw = smallp.tile([128, S, H], F32, tag="araw")
                    ve.tensor_reduce(
                        araw[:], zr[:].rearrange("p s (h c) -> p s h c", c=C),
                        AX.X, OP.add)
                    ve.tensor_tensor(
                        araw[:], araw[:],
                        _mid_bcast(hbias_sb[:, k * H:(k + 1) * H], S, 1), OP.add)
                    a_t = smallp.tile([128, S, H], F32, tag="a_t")
                    # softplus(x) = ln(exp(x) + 1)
                    se.activation(a_t[:], araw[:], AF.Exp)
                    se.activation(a_t[:], a_t[:], AF.Ln, bias=1.0)
                    a_bf = smallp.tile([128, S, H], BF16, tag="a_bf")
                    ve.tensor_scalar(a_bf[:], a_t[:], 1e-6, None, OP.add)
                    sy.dma_start(a_hbm[:, t0 * H:(t0 + S) * H],
                                 a_bf[:].rearrange("p s h -> p (s h)"))
                    pst = gathp.tile([128, S, BLK], BF16, tag="pst")
                    sy.dma_start(pst[:].rearrange("p s w -> p (s w)"),
                                 pstat_in[:, t0 * BLK:(t0 + S) * BLK])
                    for b in ch["blocks"]:
                        psd = psB.tile([128, H], F32, tag="psd")
                        first = True
                        for half in (0, 1):
                            s0 = ch["slot"][(b, half)]
                            for t in range(lay["ntile"][(b, half)]):
                                te.matmul(psd[:, :], pst[:, s0 + t, :],
                                          a_bf[:, s0 + t, :],
                                          start=first, stop=False,
                                          skip_group_check=True)
                                first = False
                        nb = min(BLK, npc - b * BLK)
                        ve.tensor_copy(deg_sb[:nb, b, :], psd[:nb, :])

                dinv_sb = holdp.tile([128, nblk, H], F32, tag="dinv")
                ve.reciprocal(dinv_sb[:], deg_sb[:])
                se.activation(dinv_sb[:], dinv_sb[:], AF.Sqrt)
                GB = 8
                for g0 in range(0, nblk, GB):
                    gn = min(GB, nblk - g0)
                    hgrp = statep.tile([128, GB, DH], F32, tag="ht_h")
                    sy.dma_start(hgrp[:, :gn, :],
                                 h_hbm[g0 * BLK:(g0 + gn) * BLK, :]
                                 .rearrange("(g p) d -> p g d", p=BLK))
                    htg = statep.tile([128, GB, DH], BF16, tag="ht_o")
                    ve.tensor_tensor(
                        htg[:, :gn, :].rearrange("p g (h c) -> p g h c", c=C),
                        hgrp[:, :gn, :].rearrange("p g (h c) -> p g h c", c=C),
                        dinv_sb[:, g0:g0 + gn, :].broadcast_to([128, gn, H, C]),
                        OP.mult)
                    sy.dma_start(ht_own[g0 * BLK:(g0 + gn) * BLK, :]
                                 .rearrange("(g p) d -> p g d", p=BLK),
                                 htg[:, :gn, :])
                gp.collective_compute(
                    "AllGather", OP.bypass,
                    replica_groups=[list(range(NCORES))],
                    ins=[ht_own[0:npc, :]], outs=[ht_full[:]],
                )

                for ci, ch in enumerate(lay["chunks"]):
                    t0, S, nlo = ch["t0"], ch["S"], ch["nlo"]
                    rc0, loW, hiW, cc0, cW = lay["chunk_idx"][ci]
                    hr = gathp.tile([128, S, DH], BF16, tag="g_a")
                    q = 3 * ci + 1
                    if loW:
                        gp.dma_gather(hr[:, :nlo, :], ht_full[0:LIM, :],
                                      rowi_sb[:, rc0:rc0 + loW], loW * 16, loW * 16,
                                      DH, single_packet=False, queue_num=q % 4)
                    if hiW:
                        gp.dma_gather(hr[:, nlo:S, :],
                                      ht_full[LIM:n_nodes, :],
                                      rowi_sb[:, rc0 + loW:rc0 + loW + hiW],
                                      hiW * 16, hiW * 16, DH,
                                      single_packet=False, queue_num=(q + 1) % 4)
                    a_t = smallp.tile([128, S, H], BF16, tag="a_t")
                    sy.dma_start(a_t[:].rearrange("p s h -> p (s h)"),
                                 a_hbm[:, t0 * H:(t0 + S) * H])
                    m_t = hr
                    ve.tensor_tensor(m_t[:].rearrange("p s (h c) -> p s h c", c=C),
                                     hr[:].rearrange("p s (h c) -> p s h c", c=C),
                                     a_t[:].broadcast_to([128, S, H, C]), OP.mult)
                    pst = gathp.tile([128, S, BLK], BF16, tag="pst")
                    sy.dma_start(pst[:].rearrange("p s w -> p (s w)"),
                                 pstat_in[:, t0 * BLK:(t0 + S) * BLK])
                    for b in ch["blocks"]:
                        psh = psA.tile([128, DH], F32, tag="psh")
                        first = True
                        for half in (0, 1):
                            s0 = ch["slot"][(b, half)]
                            for t in range(lay["ntile"][(b, half)]):
                                te.matmul(psh[:, :], pst[:, s0 + t, :],
                                          m_t[:, s0 + t, :],
                                          start=first, stop=False,
                                          skip_group_check=True)
                                first = False
                        hn = statep.tile([128, DH], F32, tag="hn")
                        ve.tensor_tensor(hc(hn[:, :]), hc(psh[:, :]),
                                         dinv_sb[:, b, :].broadcast_to([128, H, C]),
                                         OP.mult)
                        sy.dma_start(h_hbm[b * BLK:(b + 1) * BLK, :], hn[:, :])

                # grouped z/g update over node blocks
                for g0 in range(0, nblk, GB):
                    gn = min(GB, nblk - g0)
                    def grp(t):
                        return t[:, :gn, :]

                    def dram_grp(dt):
                        return (dt[g0 * BLK:(g0 + gn) * BLK, :]
                                .rearrange("(g p) d -> p g d", p=BLK))

                    def hcg(apv):
                        return apv[:, :gn, :].rearrange(
                            "p g (h c) -> p g h c", c=C)

                    hng = statep.tile([128, GB, DH], F32, tag="ht_h")
                    sy.dma_start(grp(hng), dram_grp(h_hbm))
                    e1 = statep.tile([128, GB, DH], F32, tag="tl_e1")
                    elu_(grp(e1), grp(hng), statep, "tl1")
                    ve.tensor_tensor(
                        grp(e1), grp(e1),
                        _mid_bcast(hatth_sb[:, (k - 1) * DH:k * DH], gn, 1),
                        OP.mult)
                    gv = statep.tile([128, GB, H], F32, tag="tl_gv")
                    ve.tensor_reduce(gv[:, :gn, :], hcg(e1), AX.X, OP.add)
                    zsog = statep.tile([128, GB, DH], BF16, tag="tl_zso")
                    sy.dma_start(grp(zsog), dram_grp(zs_own))
                    e2 = statep.tile([128, GB, DH], F32, tag="tl_e1")
                    elu_(grp(e2), grp(zsog), statep, "tl1")
                    ve.tensor_tensor(
                        grp(e2), grp(e2),
                        _mid_bcast(hattz_sb[:, (k - 1) * DH:k * DH], gn, 1),
                        OP.mult)
                    gv2 = statep.tile([128, GB, H], F32, tag="tl_gv2")
                    ve.tensor_reduce(gv2[:, :gn, :], hcg(e2), AX.X, OP.add)
                    ve.tensor_tensor(gv[:, :gn, :], gv[:, :gn, :],
                                     gv2[:, :gn, :], OP.add)
                    ve.tensor_tensor(
                        gv[:, :gn, :], gv[:, :gn, :],
                        _mid_bcast(hbias_sb[:, k * H:(k + 1) * H], gn, 1),
                        OP.add)
                    zbg = statep.tile([128, GB, DH], F32, tag="tl_zb")
                    sy.dma_start(grp(zbg), dram_grp(z_hbm))
                    hgx = statep.tile([128, GB, DH], F32, tag="tl_e1")
                    ve.tensor_tensor(hcg(hgx), hcg(hng),
                                     gv[:, :gn, :].broadcast_to([128, gn, H, C]),
                                     OP.mult)
                    ve.tensor_tensor(grp(zbg), grp(zbg), grp(hgx), OP.add)
                    sy.dma_start(dram_grp(z_hbm), grp(zbg))
                    zsg = statep.tile([128, GB, DH], BF16, tag="tl_zso")
                    ve.tensor_scalar(grp(zsg), grp(zbg), _decay(k), None, OP.mult)
                    sy.dma_start(dram_grp(zs_own), grp(zsg))

            # =========== classifier ===========
            for b in range(nblk):
                nb = min(BLK, npc - b * BLK)
                zb = statep.tile([128, DH], F32, tag="cl_z")
                if nb < BLK:
                    ve.memset(zb[:, :], 0.0)
                sy.dma_start(zb[:nb], z_hbm[b * BLK:b * BLK + nb, :])
                elu_(zb[:], zb[:], statep, "clelu")
                pst_ = psA.tile([128, BLK], F32, tag="mm")
                te.matmul(pst_[:, :], zb[:, :], ident_sb[:, :],
                          is_transpose=True, start=True, stop=True)
                zt = statep.tile([128, BLK], F32, tag="cl_zt")
                ve.tensor_copy(zt[:, :], pst_[:, :])
                pso = psA.tile([DOUT, BLK], F32, tag="mm")
                te.matmul(pso[:, :nb], wout_sb[:], zt[:, :nb], start=True,
                          stop=True)
                ob = statep.tile([DOUT, BLK], F32, tag="cl_o")
                ve.tensor_copy(ob[:, :nb], pso[:, :nb])
                sy.dma_start(out_ext[:, b * BLK:b * BLK + nb], ob[:, :nb])

    nc.compile()
    return nc


# ---------------------------------------------------------------------------
# Entry point
# ---------------------------------------------------------------------------

def kernel(**inputs):
    x = np.asarray(inputs["x"], np.float32)
    edge_index = np.asarray(inputs["edge_index"])
    npc = N // NCORES

    cores = _preprocess(edge_index, npc, N)
    lay, per_core = _unify(cores)
    nc = _build(lay, npc, N)

    hop_att0 = np.asarray(inputs["hop_att0"], np.float32)
    hop_atts = np.asarray(inputs["hop_atts"], np.float32)
    atts = np.asarray(inputs["atts"], np.float32)
    hop_biases = np.asarray(inputs["hop_biases"], np.float32)

    rep = lambda v: np.tile(np.asarray(v, np.float32).reshape(1, -1), (128, 1))
    att0_rep = rep(hop_att0)
    attsk_rep = rep(atts[:KHOPS].reshape(KHOPS, DH))
    hatt_h_rep = rep(hop_atts[:KHOPS, :, :C].reshape(KHOPS, DH))
    hatt_z_rep = rep(hop_atts[:KHOPS, :, C:].reshape(KHOPS, DH))
    hbias_rep = rep(hop_biases[:KHOPS + 1])
    b0_col = np.asarray(inputs["b0"], np.float32).reshape(DH, 1)
    b1_col = np.asarray(inputs["b1"], np.float32).reshape(DH, 1)
    ident = np.eye(128, dtype=np.float32)

    in_maps = []
    for r, u in enumerate(per_core):
        in_maps.append({
            "xT": np.ascontiguousarray(x[r * npc:(r + 1) * npc].T),
            "W0": np.asarray(inputs["W0"], np.float32),
            "W1": np.asarray(inputs["W1"], np.float32),
            "Wout": np.asarray(inputs["Wout"], np.float32),
            "b0_col": b0_col, "b1_col": b1_col, "ident": ident,
            "att0_rep": att0_rep, "attsk_rep": attsk_rep.astype(ml_dtypes.bfloat16),
            "hatt_h_rep": hatt_h_rep, "hatt_z_rep": hatt_z_rep,
            "hbias_rep": hbias_rep,
            "pstat": u["pstat"],
            "row_idx": (u["row_idx"] if u["row_idx"].shape[1]
                        else np.zeros((128, 1), np.int16)),
            "col_idx": (u["col_idx"] if u["col_idx"].shape[1]
                        else np.zeros((128, 1), np.int16)),
        })

    if os.environ.get("KERNEL_SIM"):
        import concourse.bass_interp as bass_interp
        sim = bass_interp.MultiCoreSim(nc, NCORES)
        for r in range(NCORES):
            for k, v in in_maps[r].items():
                sim.cores[r].tensor(k)[:] = v
        sim.simulate()
        global _LAST_SIM
        _LAST_SIM = sim
        outs = [np.array(sim.cores[r].mem_tensor("out")) for r in range(NCORES)]
    else:
        from concourse.bass_utils import run_bass_kernel_spmd
        res = run_bass_kernel_spmd(nc, in_maps, list(range(NCORES)),
                                   trace=bool(os.environ.get("KERNEL_TRACE")))
        if os.environ.get("KERNEL_TRACE") and res.exec_time_ns:
            print(f"HW exec time: {res.exec_time_ns} ns")
        outs = [res.results[r]["out"] for r in range(NCORES)]

    out = np.concatenate([o.T for o in outs], axis=0)  # [N, DOUT]
    out = out + np.asarray(inputs["bout"], np.float32)[None, :]
    return out.astype(np.float32)



# revision 8
# speedup vs baseline: 1.2231x; 1.2231x over previous
"""AERO-GNN forward pass on 8 TRN2 NeuronCores (Bass/Tile).

Sharding: edges partitioned by target-node range; core r owns target nodes
[r*NPC, (r+1)*NPC) and all edges pointing at them, so deg/h_new scatters are
core-local. Per hop, the two row-indexed tables (z_scale and dinv*h) are
replicated as bf16 via AllGather; row gathers use SWDGE dma_gather on 4
queues; segment sums (deg, h_new) run on the TensorEngine as per-tile
one-hot matmuls accumulated in PSUM node-blocks. All cores execute one SPMD
instruction stream; per-core structure differences are absorbed by padding
tile counts to cross-core maxima (padded tiles carry all-zero one-hots).
"""
import os
import sys

sys.path.insert(0, "/opt/trn_rl_repo")

import ml_dtypes
import numpy as np

import concourse.bacc as bacc
import concourse.bass as bass
import concourse.mybir as mybir
import concourse.tile as tile
from concourse import library_config

F32 = mybir.dt.float32
BF16 = mybir.dt.bfloat16
I16 = mybir.dt.int16
AX = mybir.AxisListType
OP = mybir.AluOpType
AF = mybir.ActivationFunctionType

# Problem constants (hardcoded per harness contract).
N, E = 50000, 800000
H, C = 8, 16
KHOPS = 4
DIN, DH, DOUT = 256, 128, 40
LAMBD = 1.0
NCORES = 8

IDX_LIMIT = 32768   # int16 gather index limit -> lo/hi row split
BLK = 128           # target-node block width (PSUM partitions)
BLK_PER_CHUNK = 2   # node blocks per gather chunk


def _decay(k):
    return float(np.log(LAMBD / (k + 1) + (1 + 1e-06)))


def _mid_bcast(ap, count, pos):
    """Insert a step-0 (broadcast) dim of `count` at position `pos`."""
    new_ap = [list(d) for d in ap.ap]
    new_ap = new_ap[:pos] + [[0, count]] + new_ap[pos:]
    return bass.AP(ap.tensor, ap.offset, new_ap)


def _wrap_idx(idx_flat):
    """[n] (n%16==0) -> [128, n/16] int16; slot i -> (part i%16, col i//16),
    replicated into all 8 Q7 groups."""
    n = idx_flat.shape[0]
    w = idx_flat.reshape(n // 16, 16).T.astype(np.int16)
    return np.tile(w, (8, 1))



def _patch_tile_swdge_sems():
    """Make Tile's DMASW semaphore lanes queue-aware: SWDGE queue q owns
    sems {2q, 2q+1}. Without this, round-robin assignment hands one sem to
    instructions on different SWDGE queues, which the HW/sim reject."""
    import concourse.tile_sem_assignment as tsa
    if getattr(tsa.TileClockTick, "_swdge_qpatched", False):
        return
    orig = tsa.TileClockTick._assign_tick

    def patched(self, inst):
        try:
            is_pool_dma = (isinstance(inst, tsa.DMAInst)
                           and inst.engine == mybir.EngineType.Pool)
        except Exception:
            is_pool_dma = False
        if is_pool_dma:
            q = int(getattr(inst, "queue_num", 0) or 0) % 4
            tog = self.__dict__.setdefault("_qtog", {})
            t = tog.get(q, 0)
            tog[q] = t ^ 1
            self.next_sw_dma_idx = 2 * q + t
        return orig(self, inst)

    tsa.TileClockTick._assign_tick = patched
    tsa.TileClockTick._swdge_qpatched = True


# ---------------------------------------------------------------------------
# Host-side static preprocessing
# ---------------------------------------------------------------------------

def _preprocess(edge_index, npc, n_nodes):
    """Per-core edge structure: col-sorted edges grouped into node blocks,
    split lo/hi by row id, cut into 128-edge tiles. Returns per-core dicts
    with per-(chunk, block, half) tile groups."""
    row = np.asarray(edge_index[0], dtype=np.int64)
    col = np.asarray(edge_index[1], dtype=np.int64)
    loops = np.arange(n_nodes, dtype=np.int64)
    row = np.concatenate([row, loops])
    col = np.concatenate([col, loops])

    nblk = (npc + BLK - 1) // BLK
    nchunk = (nblk + BLK_PER_CHUNK - 1) // BLK_PER_CHUNK
    cores = []
    for r in range(NCORES):
        lo_n = r * npc
        sel = (col >= lo_n) & (col < lo_n + npc)
        er = row[sel]
        ec = col[sel] - lo_n
        order = np.argsort(ec, kind="stable")
        er, ec = er[order], ec[order]
        groups = {}  # (blk, half) -> list of (rows[128], colloc[128], mask[128])
        for b in range(nblk):
            for half in (0, 1):
                m = (ec // BLK == b)
                m &= (er < IDX_LIMIT) if half == 0 else (er >= IDX_LIMIT)
                rr = er[m]
                cc = ec[m] - b * BLK
                tiles = []
                for i in range(0, max(len(rr), 1), 128):
                    pr = np.zeros(128, np.int64)
                    pc = np.zeros(128, np.int64)
                    pm = np.zeros(128, np.float32)
                    n_e = min(128, len(rr) - i)
                    if n_e > 0:
                        pr[:n_e] = rr[i:i + n_e]
                        pc[:n_e] = cc[i:i + n_e]
                        pm[:n_e] = 1.0
                    tiles.append((pr, pc, pm))
                    if len(rr) == 0:
                        break
                if len(rr) == 0:
                    tiles = []
                groups[(b, half)] = tiles
        cores.append(dict(groups=groups, nblk=nblk, nchunk=nchunk))
    return cores


def _unify(cores):
    """Pad tile counts to cross-core maxima so all cores share one layout.

    Returns (layout, per_core) where layout drives the instruction stream and
    per_core holds the data arrays (pstat, row/col idx)."""
    nblk = cores[0]["nblk"]
    nchunk = cores[0]["nchunk"]
    ntile = {}  # (blk, half) -> padded count
    for b in range(nblk):
        for half in (0, 1):
            ntile[(b, half)] = max(len(c["groups"][(b, half)]) for c in cores)

    # chunk layout: for chunk ci covering blocks [b0, b1):
    #   slots = [lo tiles of b0..b1-1] ++ [hi tiles of b0..b1-1]
    chunks = []   # (t0, S, n_lo_tiles, blocks, tile_slot[(b,half)] -> slot0)
    t_base = 0
    for ci in range(nchunk):
        b0 = ci * BLK_PER_CHUNK
        b1 = min(b0 + BLK_PER_CHUNK, nblk)
        slotmap = {}
        s = 0
        for half in (0, 1):
            for b in range(b0, b1):
                slotmap[(b, half)] = s
                s += ntile[(b, half)]
        n_lo = sum(ntile[(b, 0)] for b in range(b0, b1))
        chunks.append(dict(t0=t_base, S=s, nlo=n_lo, blocks=list(range(b0, b1)),
                           slot=slotmap))
        t_base += s
    T = t_base

    layout = dict(nblk=nblk, nchunk=nchunk, T=T, chunks=chunks, ntile=ntile)

    per_core = []
    for c in cores:
        pstat = np.zeros((T, 128, BLK), np.float32)
        rows = np.zeros((T, 128), np.int64)
        colg = np.zeros((T, 128), np.int64)
        for ch in chunks:
            for (b, half), s0 in ch["slot"].items():
                tiles = c["groups"][(b, half)]
                for t, (pr, pc, pm) in enumerate(tiles):
                    tt = ch["t0"] + s0 + t
                    rows[tt] = pr
                    colg[tt] = pc + b * BLK
                    pstat[tt, np.arange(128), pc] = pm
        # row idx arrays: per chunk [lo slots][hi slots]
        rowi_parts, coli_parts = [], []
        chunk_idx_meta = []
        rcw = ccw = 0
        for ch in chunks:
            t0, S, nlo = ch["t0"], ch["S"], ch["nlo"]
            r_lo = rows[t0:t0 + nlo].reshape(-1)
            r_hi = np.maximum(rows[t0 + nlo:t0 + S].reshape(-1) - IDX_LIMIT, 0)
            cg = colg[t0:t0 + S].reshape(-1)
            lo_w = _wrap_idx(r_lo) if nlo else np.zeros((128, 0), np.int16)
            hi_w = (_wrap_idx(r_hi) if (S - nlo) else np.zeros((128, 0), np.int16))
            c_w = _wrap_idx(cg)
            chunk_idx_meta.append((rcw, lo_w.shape[1], hi_w.shape[1],
                                   ccw, c_w.shape[1]))
            rcw += lo_w.shape[1] + hi_w.shape[1]
            ccw += c_w.shape[1]
            rowi_parts += [lo_w, hi_w]
            coli_parts += [c_w]
        per_core.append(dict(
            pstat=np.ascontiguousarray(
                pstat.transpose(1, 0, 2).reshape(128, T * BLK)
            ).astype(ml_dtypes.bfloat16),
            row_idx=np.concatenate(rowi_parts, axis=1).astype(np.int16),
            col_idx=np.concatenate(coli_parts, axis=1).astype(np.int16),
        ))
    layout["chunk_idx"] = chunk_idx_meta
    layout["row_idx_w"] = per_core[0]["row_idx"].shape[1]
    layout["col_idx_w"] = per_core[0]["col_idx"].shape[1]
    return layout, per_core


# ---------------------------------------------------------------------------
# Device graph (SPMD; one instruction stream for all 8 cores)
# ---------------------------------------------------------------------------

def _build(lay, npc, n_nodes):
    nblk = lay["nblk"]
    T = lay["T"]
    LIM = min(IDX_LIMIT, n_nodes)

    _patch_tile_swdge_sems()
    nc = bacc.Bacc("TRN2", target_bir_lowering=False, debug=False,
                   num_swdge_queues=4)

    dram_in = lambda name, shape, dt: nc.dram_tensor(name, shape, dt,
                                                     kind="ExternalInput")
    xT = dram_in("xT", [DIN, npc], F32)
    W0 = dram_in("W0", [DIN, DH], F32)
    W1 = dram_in("W1", [DH, DH], F32)
    Wout = dram_in("Wout", [DH, DOUT], F32)
    b0_col = dram_in("b0_col", [DH, 1], F32)
    b1_col = dram_in("b1_col", [DH, 1], F32)
    ident_in = dram_in("ident", [128, 128], F32)
    att0_rep = dram_in("att0_rep", [128, DH], F32)
    attsk_rep = dram_in("attsk_rep", [128, KHOPS * DH], BF16)
    hatt_h_rep = dram_in("hatt_h_rep", [128, KHOPS * DH], F32)
    hatt_z_rep = dram_in("hatt_z_rep", [128, KHOPS * DH], F32)
    hbias_rep = dram_in("hbias_rep", [128, (KHOPS + 1) * H], F32)
    pstat_in = dram_in("pstat", [128, T * BLK], BF16)
    row_idx_in = dram_in("row_idx", [128, max(lay["row_idx_w"], 1)], I16)
    col_idx_in = dram_in("col_idx", [128, max(lay["col_idx_w"], 1)], I16)

    out_ext = nc.dram_tensor("out", [DOUT, npc], F32, kind="ExternalOutput")
    hdbg = nc.dram_tensor("hdbg", [npc, DH], F32, kind="ExternalOutput") if os.environ.get("KERNEL_DEBUG") else None

    npc_pad = nblk * BLK
    h_hbm = nc.dram_tensor("h_hbm", [npc_pad, DH], F32)
    z_hbm = nc.dram_tensor("z_hbm", [npc_pad, DH], F32)
    zs_own = nc.dram_tensor("zs_own", [npc_pad, DH], BF16)
    ht_own = nc.dram_tensor("ht_own", [npc_pad, DH], BF16)
    zs_full = nc.dram_tensor("zs_full", [n_nodes, DH], BF16, addr_space="Shared")
    ht_full = nc.dram_tensor("ht_full", [n_nodes, DH], BF16, addr_space="Shared")

    with tile.TileContext(nc) as tc:
        with (
            tc.tile_pool(name="const", bufs=1) as constp,
            tc.tile_pool(name="state", bufs=2) as statep,
            tc.tile_pool(name="gath", bufs=2) as gathp,
            tc.tile_pool(name="zcg", bufs=2) as zcp,
            tc.tile_pool(name="work", bufs=2) as workp,
            tc.tile_pool(name="small", bufs=2) as smallp,
            tc.tile_pool(name="hold", bufs=1) as holdp,
            tc.tile_pool(name="psA", bufs=2, space="PSUM") as psA,
            tc.tile_pool(name="psB", bufs=2, space="PSUM") as psB,
        ):
            gp, ve, se, te = nc.gpsimd, nc.vector, nc.scalar, nc.tensor
            sy = nc.sync

            gp.load_library(library_config.mlp)

            # Round-robin SWDGE gathers over all 4 queues in <=2048-idx
            # pieces: two pieces fit in a queue's 4096-descriptor ring, and
            # 4 active queues give ~4x the random-read bandwidth of one
            # (each SDMA engine keeps one outstanding read per queue).
            qctr = [0]

            def emit_gather(dst, tab, idx_sb_, c0, W, slot0):
                """Gather W*16 rows of tab into dst[:, slot0:...]; W cols of
                16 idxs, W % 8 == 0."""
                off = 0
                while off < W:
                    w = min(128, W - off)
                    s0 = slot0 + off // 8
                    gp.dma_gather(dst[:, s0:s0 + w // 8, :], tab,
                                  idx_sb_[:, c0 + off:c0 + off + w],
                                  w * 16, w * 16, DH, single_packet=False,
                                  queue_num=qctr[0] % 4)
                    qctr[0] += 1
                    off += w

            def ctile(shape, dt, tag, src):
                t = constp.tile(shape, dt, tag=tag)
                sy.dma_start(t[:], src[:])
                return t

            w0_sb = constp.tile([128, 2, DH], F32, tag="w0")
            for kc in range(2):
                sy.dma_start(w0_sb[:, kc, :], W0[kc * 128:(kc + 1) * 128, :])
            w1_sb = ctile([DH, DH], F32, "w1", W1)
            wout_sb = ctile([DH, DOUT], F32, "wout", Wout)
            rowi_sb = ctile([128, max(lay["row_idx_w"], 1)], I16, "rowi",
                            row_idx_in)
            coli_sb = ctile([128, max(lay["col_idx_w"], 1)], I16, "coli",
                            col_idx_in)
            b0_sb = ctile([DH, 1], F32, "b0", b0_col)
            b1_sb = ctile([DH, 1], F32, "b1", b1_col)
            ident_sb = ctile([128, 128], F32, "ident", ident_in)
            att0_sb = ctile([128, DH], F32, "att0", att0_rep)
            attsk_sb = ctile([128, KHOPS * DH], BF16, "attsk", attsk_rep)
            hatth_sb = ctile([128, KHOPS * DH], F32, "hatth", hatt_h_rep)
            hattz_sb = ctile([128, KHOPS * DH], F32, "hattz", hatt_z_rep)
            hbias_sb = ctile([128, (KHOPS + 1) * H], F32, "hbias", hbias_rep)

            def elu_(dst, src, pool, tag):
                # elu(x) = (max(x,0) - 1) + min(e^x, 1); inputs are O(1) so
                # the direct Exp cannot overflow. 1 ACT + 2 DVE passes.
                p = src.shape[0]
                rest = list(src.shape[1:])
                mn = pool.tile([128] + rest, F32, tag=tag + "_mn")
                ex = pool.tile([128] + rest, F32, tag=tag + "_ex")
                se.activation(ex[:p], src, AF.Exp)
                ve.tensor_scalar(mn[:p], src, 0.0, -1.0, OP.max, OP.add)
                ve.scalar_tensor_tensor(dst, ex[:p], 1.0, mn[:p], OP.min, OP.add)

            def hc(apv):
                return apv.rearrange("p (h c) -> p h c", c=C)

            # =========== MLP + k=0 ===========
            for b in range(nblk):
                nb = min(BLK, npc - b * BLK)
                xt_sb = statep.tile([128, 2, BLK], F32, tag="xt")
                for kc in range(2):
                    sy.dma_start(xt_sb[:, kc, :nb],
                                 xT[kc * 128:(kc + 1) * 128, b * BLK:b * BLK + nb])
                ps = psA.tile([128, BLK], F32, tag="mm")
                for kc in range(2):
                    te.matmul(ps[:, :nb], w0_sb[:, kc, :], xt_sb[:, kc, :nb],
                              start=(kc == 0), stop=(kc == 1))
                h1t = statep.tile([128, BLK], F32, tag="h1t")
                ve.tensor_tensor(h1t[:, :nb], ps[:, :nb],
                                 b0_sb[:, 0:1].broadcast_to([DH, nb]), OP.add)
                elu_(h1t[:, :nb], h1t[:, :nb], statep, "melu")
                ps2 = psA.tile([128, BLK], F32, tag="mm")
                te.matmul(ps2[:, :nb], w1_sb[:], h1t[:, :nb], start=True, stop=True)
                h2t = statep.tile([128, BLK], F32, tag="h2t")
                ve.tensor_tensor(h2t[:, :nb], ps2[:, :nb],
                                 b1_sb[:, 0:1].broadcast_to([DH, nb]), OP.add)
                ps3 = psA.tile([128, BLK], F32, tag="mm")
                te.matmul(ps3[:, :], h2t[:, :], ident_sb[:, :],
                          is_transpose=True, start=True, stop=True)
                hfin = statep.tile([128, DH], F32, tag="hfin")
                ve.tensor_copy(hfin[:, :], ps3[:, :])
                sy.dma_start(h_hbm[b * BLK:(b + 1) * BLK, :], hfin[:, :])
                if hdbg is not None:
                    sy.dma_start(hdbg[b * BLK:b * BLK + nb, :], hfin[:nb])

            # k=0 gate/update, grouped over node blocks
            GB = 8
            for g0i in range(0, nblk, GB):
                gn = min(GB, nblk - g0i)

                def grp0(t):
                    return t[:, :gn, :]

                def dram_grp0(dt):
                    return (dt[g0i * BLK:(g0i + gn) * BLK, :]
                            .rearrange("(g p) d -> p g d", p=BLK))

                def hcg0(apv):
                    return apv[:, :gn, :].rearrange("p g (h c) -> p g h c", c=C)

                hng = statep.tile([128, GB, DH], F32, tag="ht_h")
                sy.dma_start(grp0(hng), dram_grp0(h_hbm))
                eh = statep.tile([128, GB, DH], F32, tag="tl_e1")
                elu_(grp0(eh), grp0(hng), statep, "tl1")
                ve.tensor_tensor(grp0(eh), grp0(eh),
                                 _mid_bcast(att0_sb[:, :], gn, 1), OP.mult)
                g0v = statep.tile([128, GB, H], F32, tag="tl_gv")
                ve.tensor_reduce(g0v[:, :gn, :], hcg0(eh), AX.X, OP.add)
                ve.tensor_tensor(g0v[:, :gn, :], g0v[:, :gn, :],
                                 _mid_bcast(hbias_sb[:, 0:H], gn, 1), OP.add)
                ztg = statep.tile([128, GB, DH], F32, tag="tl_zb")
                ve.tensor_tensor(hcg0(ztg), hcg0(hng),
                                 g0v[:, :gn, :].broadcast_to([128, gn, H, C]),
                                 OP.mult)
                sy.dma_start(dram_grp0(z_hbm), grp0(ztg))
                zsg = statep.tile([128, GB, DH], BF16, tag="tl_zso")
                ve.tensor_scalar(grp0(zsg), grp0(ztg), _decay(0), None, OP.mult)
                sy.dma_start(dram_grp0(zs_own), grp0(zsg))

            # =========== hops ===========
            for k in range(1, KHOPS + 1):
                gp.collective_compute(
                    "AllGather", OP.bypass,
                    replica_groups=[list(range(NCORES))],
                    ins=[zs_own[0:npc, :]], outs=[zs_full[:]],
                )

                a_all = holdp.tile([128, T, H], BF16, tag="a_all")
                deg_sb = holdp.tile([128, nblk, H], F32, tag="deg")

                # zc gathers don't depend on the AllGather; prefetch them so
                # the queues stay busy while zs_full lands.
                zc_ring = {}

                def issue_zc(cj):
                    if cj >= len(lay["chunks"]):
                        return
                    chj = lay["chunks"][cj]
                    _, _, _, cc0j, cWj = lay["chunk_idx"][cj]
                    zcj = zcp.tile([128, chj["S"], DH], BF16, tag="g_zc")
                    emit_gather(zcj, zs_own[:, :], coli_sb, cc0j, cWj, 0)
                    zc_ring[cj] = zcj

                issue_zc(0)
                for ci, ch in enumerate(lay["chunks"]):
                    t0, S, nlo = ch["t0"], ch["S"], ch["nlo"]
                    rc0, loW, hiW, cc0, cW = lay["chunk_idx"][ci]
                    zr = gathp.tile([128, S, DH], BF16, tag="g_a")
                    if loW:
                        emit_gather(zr, zs_full[0:LIM, :], rowi_sb, rc0, loW, 0)
                    if hiW:
                        emit_gather(zr, zs_full[LIM:n_nodes, :], rowi_sb,
                                    rc0 + loW, hiW, nlo)
                    issue_zc(ci + 1)
                    zc = zc_ring.pop(ci)
                    ve.tensor_tensor(zr[:], zr[:], zc[:], OP.add)
                    tmp = workp.tile([128, S, DH], BF16, tag="welu")
                    se.activation(tmp[:], zr[:], AF.Exp)
                    ve.tensor_scalar(zr[:], zr[:], 0.0, -1.0, OP.max, OP.add)
                    ve.scalar_tensor_tensor(zr[:], tmp[:], 1.0, zr[:],
                                            OP.min, OP.add)
                    ve.tensor_tensor(
                        zr[:], zr[:],
                        _mid_bcast(attsk_sb[:, (k - 1) * DH:k * DH], S, 1),
                        OP.mult)
                    araw = smallp.tile([128, S, H], F32, tag="araw")
                    ve.tensor_reduce(
                        araw[:], zr[:].rearrange("p s (h c) -> p s h c", c=C),
                        AX.X, OP.add)
                    ve.tensor_tensor(
                        araw[:], araw[:],
                        _mid_bcast(hbias_sb[:, k * H:(k + 1) * H], S, 1), OP.add)
                    a_t = smallp.tile([128, S, H], F32, tag="a_t")
                    # softplus(x) = ln(exp(x) + 1)
                    se.activation(a_t[:], araw[:], AF.Exp)
                    se.activation(a_t[:], a_t[:], AF.Ln, bias=1.0)
                    a_v = a_all[:, t0:t0 + S, :]
                    ve.tensor_scalar(a_v, a_t[:], 1e-6, None, OP.add)
                    pst = gathp.tile([128, S, BLK], BF16, tag="pst")
                    sy.dma_start(pst[:].rearrange("p s w -> p (s w)"),
                                 pstat_in[:, t0 * BLK:(t0 + S) * BLK])
                    for b in ch["blocks"]:
                        psd = psB.tile([128, H], F32, tag="psd")
                        first = True
                        for half in (0, 1):
                            s0 = ch["slot"][(b, half)]
                            for t in range(lay["ntile"][(b, half)]):
                                te.matmul(psd[:, :], pst[:, s0 + t, :],
                                          a_all[:, t0 + s0 + t, :],
                                          start=first, stop=False,
                                          skip_group_check=True)
                                first = False
                        nb = min(BLK, npc - b * BLK)
                        ve.tensor_copy(deg_sb[:nb, b, :], psd[:nb, :])

                dinv_sb = holdp.tile([128, nblk, H], F32, tag="dinv")
                ve.reciprocal(dinv_sb[:], deg_sb[:])
                se.activation(dinv_sb[:], dinv_sb[:], AF.Sqrt)
                GB = 8
                for g0 in range(0, nblk, GB):
                    gn = min(GB, nblk - g0)
                    hgrp = statep.tile([128, GB, DH], F32, tag="ht_h")
                    sy.dma_start(hgrp[:, :gn, :],
                                 h_hbm[g0 * BLK:(g0 + gn) * BLK, :]
                                 .rearrange("(g p) d -> p g d", p=BLK))
                    htg = statep.tile([128, GB, DH], BF16, tag="ht_o")
                    ve.tensor_tensor(
                        htg[:, :gn, :].rearrange("p g (h c) -> p g h c", c=C),
                        hgrp[:, :gn, :].rearrange("p g (h c) -> p g h c", c=C),
                        dinv_sb[:, g0:g0 + gn, :].broadcast_to([128, gn, H, C]),
                        OP.mult)
                    sy.dma_start(ht_own[g0 * BLK:(g0 + gn) * BLK, :]
                                 .rearrange("(g p) d -> p g d", p=BLK),
                                 htg[:, :gn, :])
                gp.collective_compute(
                    "AllGather", OP.bypass,
                    replica_groups=[list(range(NCORES))],
                    ins=[ht_own[0:npc, :]], outs=[ht_full[:]],
                )

                for ci, ch in enumerate(lay["chunks"]):
                    t0, S, nlo = ch["t0"], ch["S"], ch["nlo"]
                    rc0, loW, hiW, cc0, cW = lay["chunk_idx"][ci]
                    hr = gathp.tile([128, S, DH], BF16, tag="g_a")
                    if loW:
                        emit_gather(hr, ht_full[0:LIM, :], rowi_sb, rc0, loW, 0)
                    if hiW:
                        emit_gather(hr, ht_full[LIM:n_nodes, :], rowi_sb,
                                    rc0 + loW, hiW, nlo)
                    m_t = hr
                    ve.tensor_tensor(m_t[:].rearrange("p s (h c) -> p s h c", c=C),
                                     hr[:].rearrange("p s (h c) -> p s h c", c=C),
                                     a_all[:, t0:t0 + S, :]
                                     .broadcast_to([128, S, H, C]), OP.mult)
                    pst = gathp.tile([128, S, BLK], BF16, tag="pst")
                    sy.dma_start(pst[:].rearrange("p s w -> p (s w)"),
                                 pstat_in[:, t0 * BLK:(t0 + S) * BLK])
                    for b in ch["blocks"]:
                        psh = psA.tile([128, DH], F32, tag="psh")
                        first = True
                        for half in (0, 1):
                            s0 = ch["slot"][(b, half)]
                            for t in range(lay["ntile"][(b, half)]):
                                te.matmul(psh[:, :], pst[:, s0 + t, :],
                                          m_t[:, s0 + t, :],
                                          start=first, stop=False,
                                          skip_group_check=True)
                                first = False
                        hn = statep.tile([128, DH], F32, tag="hn")
                        ve.tensor_tensor(hc(hn[:, :]), hc(psh[:, :]),
                                         dinv_sb[:, b, :].broadcast_to([128, H, C]),
                                         OP.mult)
                        sy.dma_start(h_hbm[b * BLK:(b + 1) * BLK, :], hn[:, :])

                # grouped z/g update over node blocks
                for g0 in range(0, nblk, GB):
                    gn = min(GB, nblk - g0)
                    def grp(t):
                        return t[:, :gn, :]

                    def dram_grp(dt):
                        return (dt[g0 * BLK:(g0 + gn) * BLK, :]
                                .rearrange("(g p) d -> p g d", p=BLK))

                    def hcg(apv):
                        return apv[:, :gn, :].rearrange(
                            "p g (h c) -> p g h c", c=C)

                    hng = statep.tile([128, GB, DH], F32, tag="ht_h")
                    sy.dma_start(grp(hng), dram_grp(h_hbm))
                    e1 = statep.tile([128, GB, DH], F32, tag="tl_e1")
                    elu_(grp(e1), grp(hng), statep, "tl1")
                    ve.tensor_tensor(
                        grp(e1), grp(e1),
                        _mid_bcast(hatth_sb[:, (k - 1) * DH:k * DH], gn, 1),
                        OP.mult)
                    gv = statep.tile([128, GB, H], F32, tag="tl_gv")
                    ve.tensor_reduce(gv[:, :gn, :], hcg(e1), AX.X, OP.add)
                    zsog = statep.tile([128, GB, DH], BF16, tag="tl_zso")
                    sy.dma_start(grp(zsog), dram_grp(zs_own))
                    e2 = statep.tile([128, GB, DH], F32, tag="tl_e1")
                    elu_(grp(e2), grp(zsog), statep, "tl1")
                    ve.tensor_tensor(
                        grp(e2), grp(e2),
                        _mid_bcast(hattz_sb[:, (k - 1) * DH:k * DH], gn, 1),
                        OP.mult)
                    gv2 = statep.tile([128, GB, H], F32, tag="tl_gv2")
                    ve.tensor_reduce(gv2[:, :gn, :], hcg(e2), AX.X, OP.add)
                    ve.tensor_tensor(gv[:, :gn, :], gv[:, :gn, :],
                                     gv2[:, :gn, :], OP.add)
                    ve.tensor_tensor(
                        gv[:, :gn, :], gv[:, :gn, :],
                        _mid_bcast(hbias_sb[:, k * H:(k + 1) * H], gn, 1),
                        OP.add)
                    zbg = statep.tile([128, GB, DH], F32, tag="tl_zb")
                    sy.dma_start(grp(zbg), dram_grp(z_hbm))
                    hgx = statep.tile([128, GB, DH], F32, tag="tl_e1")
                    ve.tensor_tensor(hcg(hgx), hcg(hng),
                                     gv[:, :gn, :].broadcast_to([128, gn, H, C]),
                                     OP.mult)
                    ve.tensor_tensor(grp(zbg), grp(zbg), grp(hgx), OP.add)
                    sy.dma_start(dram_grp(z_hbm), grp(zbg))
                    zsg = statep.tile([128, GB, DH], BF16, tag="tl_zso")
                    ve.tensor_scalar(grp(zsg), grp(zbg), _decay(k), None, OP.mult)
                    sy.dma_start(dram_grp(zs_own), grp(zsg))

            # =========== classifier ===========
            for b in range(nblk):
                nb = min(BLK, npc - b * BLK)
                zb = statep.tile([128, DH], F32, tag="cl_z")
                if nb < BLK:
                    ve.memset(zb[:, :], 0.0)
                sy.dma_start(zb[:nb], z_hbm[b * BLK:b * BLK + nb, :])
                elu_(zb[:], zb[:], statep, "clelu")
                pst_ = psA.tile([128, BLK], F32, tag="mm")
                te.matmul(pst_[:, :], zb[:, :], ident_sb[:, :],
                          is_transpose=True, start=True, stop=True)
                zt = statep.tile([128, BLK], F32, tag="cl_zt")
                ve.tensor_copy(zt[:, :], pst_[:, :])
                pso = psA.tile([DOUT, BLK], F32, tag="mm")
                te.matmul(pso[:, :nb], wout_sb[:], zt[:, :nb], start=True,
                          stop=True)
                ob = statep.tile([DOUT, BLK], F32, tag="cl_o")
                ve.tensor_copy(ob[:, :nb], pso[:, :nb])
                sy.dma_start(out_ext[:, b * BLK:b * BLK + nb], ob[:, :nb])

    nc.compile()
    return nc


# ---------------------------------------------------------------------------
# Entry point
# ---------------------------------------------------------------------------

def kernel(**inputs):
    x = np.asarray(inputs["x"], np.float32)
    edge_index = np.asarray(inputs["edge_index"])
    npc = N // NCORES

    cores = _preprocess(edge_index, npc, N)
    lay, per_core = _unify(cores)
    nc = _build(lay, npc, N)

    hop_att0 = np.asarray(inputs["hop_att0"], np.float32)
    hop_atts = np.asarray(inputs["hop_atts"], np.float32)
    atts = np.asarray(inputs["atts"], np.float32)
    hop_biases = np.asarray(inputs["hop_biases"], np.float32)

    rep = lambda v: np.tile(np.asarray(v, np.float32).reshape(1, -1), (128, 1))
    att0_rep = rep(hop_att0)
    attsk_rep = rep(atts[:KHOPS].reshape(KHOPS, DH))
    hatt_h_rep = rep(hop_atts[:KHOPS, :, :C].reshape(KHOPS, DH))
    hatt_z_rep = rep(hop_atts[:KHOPS, :, C:].reshape(KHOPS, DH))
    hbias_rep = rep(hop_biases[:KHOPS + 1])
    b0_col = np.asarray(inputs["b0"], np.float32).reshape(DH, 1)
    b1_col = np.asarray(inputs["b1"], np.float32).reshape(DH, 1)
    ident = np.eye(128, dtype=np.float32)

    in_maps = []
    for r, u in enumerate(per_core):
        in_maps.append({
            "xT": np.ascontiguousarray(x[r * npc:(r + 1) * npc].T),
            "W0": np.asarray(inputs["W0"], np.float32),
            "W1": np.asarray(inputs["W1"], np.float32),
            "Wout": np.asarray(inputs["Wout"], np.float32),
            "b0_col": b0_col, "b1_col": b1_col, "ident": ident,
            "att0_rep": att0_rep, "attsk_rep": attsk_rep.astype(ml_dtypes.bfloat16),
            "hatt_h_rep": hatt_h_rep, "hatt_z_rep": hatt_z_rep,
            "hbias_rep": hbias_rep,
            "pstat": u["pstat"],
            "row_idx": (u["row_idx"] if u["row_idx"].shape[1]
                        else np.zeros((128, 1), np.int16)),
            "col_idx": (u["col_idx"] if u["col_idx"].shape[1]
                        else np.zeros((128, 1), np.int16)),
        })

    if os.environ.get("KERNEL_SIM"):
        import concourse.bass_interp as bass_interp
        sim = bass_interp.MultiCoreSim(nc, NCORES)
        for r in range(NCORES):
            for k, v in in_maps[r].items():
                sim.cores[r].tensor(k)[:] = v
        sim.simulate()
        global _LAST_SIM
        _LAST_SIM = sim
        outs = [np.array(sim.cores[r].mem_tensor("out")) for r in range(NCORES)]
    else:
        from concourse.bass_utils import run_bass_kernel_spmd
        res = run_bass_kernel_spmd(nc, in_maps, list(range(NCORES)),
                                   trace=bool(os.environ.get("KERNEL_TRACE")))
        if os.environ.get("KERNEL_TRACE") and res.exec_time_ns:
            print(f"HW exec time: {res.exec_time_ns} ns")
        outs = [res.results[r]["out"] for r in range(NCORES)]

    out = np.concatenate([o.T for o in outs], axis=0)  # [N, DOUT]
    out = out + np.asarray(inputs["bout"], np.float32)[None, :]
    return out.astype(np.float32)



# revision 31
# speedup vs baseline: 1.4536x; 1.1885x over previous
"""AERO-GNN forward pass on 8 TRN2 NeuronCores (Bass/Tile).

Sharding: edges partitioned by target-node range; core r owns target nodes
[r*NPC, (r+1)*NPC) and all edges pointing at them, so deg/h_new scatters are
core-local. Per hop, the two row-indexed tables (z_scale and dinv*h) are
replicated as bf16 via AllGather; row gathers use SWDGE dma_gather on 4
queues; segment sums (deg, h_new) run on the TensorEngine as per-tile
one-hot matmuls accumulated in PSUM node-blocks. All cores execute one SPMD
instruction stream; per-core structure differences are absorbed by padding
tile counts to cross-core maxima (padded tiles carry all-zero one-hots).
"""
import os
import sys

sys.path.insert(0, "/opt/trn_rl_repo")

import ml_dtypes
import numpy as np

import concourse.bacc as bacc
import concourse.bass as bass
import concourse.mybir as mybir
import concourse.tile as tile
from concourse import library_config

F32 = mybir.dt.float32
BF16 = mybir.dt.bfloat16
I16 = mybir.dt.int16
AX = mybir.AxisListType
OP = mybir.AluOpType
AF = mybir.ActivationFunctionType

# Problem constants (hardcoded per harness contract).
N, E = 50000, 800000
H, C = 8, 16
KHOPS = 4
DIN, DH, DOUT = 256, 128, 40
LAMBD = 1.0
NCORES = 8

IDX_LIMIT = 32768   # int16 gather index limit -> lo/hi row split
BLK = 128           # target-node block width (PSUM partitions)
BLK_PER_CHUNK = 2   # node blocks per gather chunk

# Row tables are AllGathered in two halves so the second half can overlap
# gather work: "lo" rows are (n % npc) < HALF of every core, "hi" the rest.
HALF = 3200         # 25 blocks of 128
HIW = 3050          # real hi rows per core (npc - HALF)
HIPAD = 3072        # padded hi rows per core (npc_pad - HALF)
G7 = [(0, 8), (8, 8), (16, 8), (24, 1),
      (25, 8), (33, 8), (41, 8)]  # (b0, gn) groups; none straddle block 25


def _decay(k):
    return float(np.log(LAMBD / (k + 1) + (1 + 1e-06)))


def _mid_bcast(ap, count, pos):
    """Insert a step-0 (broadcast) dim of `count` at position `pos`."""
    new_ap = [list(d) for d in ap.ap]
    new_ap = new_ap[:pos] + [[0, count]] + new_ap[pos:]
    return bass.AP(ap.tensor, ap.offset, new_ap)


def _wrap_idx(idx_flat):
    """[n] (n%16==0) -> [128, n/16] int16; slot i -> (part i%16, col i//16),
    replicated into all 8 Q7 groups."""
    n = idx_flat.shape[0]
    w = idx_flat.reshape(n // 16, 16).T.astype(np.int16)
    return np.tile(w, (8, 1))



def _patch_tile_swdge_sems():
    """Make Tile's DMASW semaphore lanes queue-aware: SWDGE queue q owns
    sems {2q, 2q+1}. Without this, round-robin assignment hands one sem to
    instructions on different SWDGE queues, which the HW/sim reject."""
    import concourse.tile_sem_assignment as tsa
    if getattr(tsa.TileClockTick, "_swdge_qpatched", False):
        return
    orig = tsa.TileClockTick._assign_tick

    def patched(self, inst):
        try:
            is_pool_dma = (isinstance(inst, tsa.DMAInst)
                           and inst.engine == mybir.EngineType.Pool)
        except Exception:
            is_pool_dma = False
        if is_pool_dma:
            q = int(getattr(inst, "queue_num", 0) or 0) % 4
            tog = self.__dict__.setdefault("_qtog", {})
            t = tog.get(q, 0)
            tog[q] = t ^ 1
            self.next_sw_dma_idx = 2 * q + t
        return orig(self, inst)

    tsa.TileClockTick._assign_tick = patched
    tsa.TileClockTick._swdge_qpatched = True


# ---------------------------------------------------------------------------
# Host-side static preprocessing
# ---------------------------------------------------------------------------

def _preprocess(edge_index, npc, n_nodes):
    """Per-core edge structure WITHOUT self-loops (handled node-locally):
    edges grouped per (chunk, half) where half is by source row
    (row % npc < HALF), sorted by (col, row), cut into 128-edge tiles that
    may span the chunk's two node blocks."""
    row = np.asarray(edge_index[0], dtype=np.int64)
    col = np.asarray(edge_index[1], dtype=np.int64)

    nblk = (npc + BLK - 1) // BLK
    nchunk = (nblk + BLK_PER_CHUNK - 1) // BLK_PER_CHUNK
    cores = []
    for r in range(NCORES):
        lo_n = r * npc
        sel = (col >= lo_n) & (col < lo_n + npc)
        er = row[sel]
        ec = col[sel] - lo_n
        groups = {}  # (chunk, half) -> list of (rows[128], colloc[128])
        for ci in range(nchunk):
            b0 = ci * BLK_PER_CHUNK
            b1 = min(b0 + BLK_PER_CHUNK, nblk)
            mc = (ec // BLK >= b0) & (ec // BLK < b1)
            for half in (0, 1):
                m = mc & (((er % npc) < HALF) if half == 0
                          else ((er % npc) >= HALF))
                rr, cc = er[m], ec[m]
                order = np.lexsort((rr, cc))
                rr, cc = rr[order], cc[order]
                tiles = []
                for i in range(0, len(rr), 128):
                    pr = np.zeros(128, np.int64)
                    pc = np.full(128, -1, np.int64)
                    n_e = min(128, len(rr) - i)
                    pr[:n_e] = rr[i:i + n_e]
                    pc[:n_e] = cc[i:i + n_e]
                    tiles.append((pr, pc))
                groups[(ci, half)] = tiles
        cores.append(dict(groups=groups, nblk=nblk, nchunk=nchunk))
    return cores


def _unify(cores):
    """Pad tile counts to cross-core maxima so all cores share one layout.

    Returns (layout, per_core) where layout drives the instruction stream and
    per_core holds the data arrays (pstat, row/col idx)."""
    nblk = cores[0]["nblk"]
    nchunk = cores[0]["nchunk"]
    ntile = {}  # (chunk, half) -> padded count
    for ci in range(nchunk):
        for half in (0, 1):
            ntile[(ci, half)] = max(len(c["groups"][(ci, half)]) for c in cores)

    chunks = []
    t_base = 0
    for ci in range(nchunk):
        b0 = ci * BLK_PER_CHUNK
        b1 = min(b0 + BLK_PER_CHUNK, nblk)
        n_lo = ntile[(ci, 0)]
        s = n_lo + ntile[(ci, 1)]
        chunks.append(dict(t0=t_base, S=s, nlo=n_lo,
                           blocks=list(range(b0, b1))))
        t_base += s
    T = t_base

    # Per (chunk, block): the union (across cores) of tile-slot ranges whose
    # edges hit that block; within a half the slots hitting a block are a
    # contiguous run because edges are col-sorted. Boundary tiles appear in
    # both blocks' lists (with different one-hot columns).
    mm_slots = {}  # (ci, b) -> sorted slot list
    for ci, ch in enumerate(chunks):
        for b in ch["blocks"]:
            slots = set()
            for c in cores:
                for half in (0, 1):
                    base = 0 if half == 0 else ch["nlo"]
                    lo_t = hi_t = None
                    for t, (pr, pc) in enumerate(c["groups"][(ci, half)]):
                        hit = np.any(pc // BLK == b)
                        if hit:
                            if lo_t is None:
                                lo_t = t
                            hi_t = t
                    if lo_t is not None:
                        slots.update(range(base + lo_t, base + hi_t + 1))
            mm_slots[(ci, b)] = sorted(slots)

    # pstat column layout: for ci, for b, for slot -> one 128-col group
    pcol = {}
    P = 0
    for ci, ch in enumerate(chunks):
        for b in ch["blocks"]:
            for s in mm_slots[(ci, b)]:
                pcol[(ci, b, s)] = P
                P += 1
    for ci, ch in enumerate(chunks):
        ch["mm"] = {b: [(s, pcol[(ci, b, s)]) for s in mm_slots[(ci, b)]]
                    for b in ch["blocks"]}
        ch["P0"] = min(pcol[(ci, b, s)] for b in ch["blocks"]
                       for s in mm_slots[(ci, b)])
        ch["P"] = sum(len(mm_slots[(ci, b)]) for b in ch["blocks"])

    layout = dict(nblk=nblk, nchunk=nchunk, T=T, chunks=chunks, ntile=ntile,
                  P=P)

    per_core = []
    npc = N // NCORES
    for c in cores:
        rows = np.zeros((T, 128), np.int64)
        cloc = np.full((T, 128), -1, np.int64)
        for ci, ch in enumerate(chunks):
            for half in (0, 1):
                base = ch["t0"] + (0 if half == 0 else ch["nlo"])
                for t, (pr, pc) in enumerate(c["groups"][(ci, half)]):
                    rows[base + t] = pr
                    cloc[base + t] = pc
        pstat = np.zeros((P, 128, BLK), np.float32)
        for ci, ch in enumerate(chunks):
            for b in ch["blocks"]:
                for (s, p) in ch["mm"][b]:
                    cc = cloc[ch["t0"] + s]
                    m = (cc // BLK == b)
                    e_idx = np.nonzero(m)[0]
                    pstat[p, e_idx, cc[e_idx] - b * BLK] = 1.0
        rowi_parts, coli_parts = [], []
        chunk_idx_meta = []
        rcw = ccw = 0
        for ch in chunks:
            t0, S, nlo = ch["t0"], ch["S"], ch["nlo"]
            # lo rows: (q, m) = divmod(row, npc); lo table idx = q*HALF + m,
            # hi table idx = q*HIW + (m - HALF); pad rows (0) clamp to 0.
            rlq, rlm = np.divmod(rows[t0:t0 + nlo].reshape(-1), npc)
            r_lo = rlq * HALF + rlm
            rhq, rhm = np.divmod(rows[t0 + nlo:t0 + S].reshape(-1), npc)
            r_hi = np.maximum(rhq * HIW + (rhm - HALF), 0)
            cg = np.maximum(cloc[t0:t0 + S].reshape(-1), 0)
            lo_w = _wrap_idx(r_lo) if nlo else np.zeros((128, 0), np.int16)
            hi_w = (_wrap_idx(r_hi) if (S - nlo) else np.zeros((128, 0), np.int16))
            c_w = _wrap_idx(cg)
            chunk_idx_meta.append((rcw, lo_w.shape[1], hi_w.shape[1],
                                   ccw, c_w.shape[1]))
            rcw += lo_w.shape[1] + hi_w.shape[1]
            ccw += c_w.shape[1]
            rowi_parts += [lo_w, hi_w]
            coli_parts += [c_w]
        per_core.append(dict(
            pstat=np.ascontiguousarray(
                pstat.transpose(1, 0, 2).reshape(128, P * BLK)
            ).astype(ml_dtypes.bfloat16),
            row_idx=np.concatenate(rowi_parts, axis=1).astype(np.int16),
            col_idx=np.concatenate(coli_parts, axis=1).astype(np.int16),
        ))
    layout["chunk_idx"] = chunk_idx_meta
    layout["row_idx_w"] = per_core[0]["row_idx"].shape[1]
    layout["col_idx_w"] = per_core[0]["col_idx"].shape[1]
    return layout, per_core


# ---------------------------------------------------------------------------
# Device graph (SPMD; one instruction stream for all 8 cores)
# ---------------------------------------------------------------------------

def _build(lay, npc, n_nodes):
    nblk = lay["nblk"]
    T = lay["T"]
    LIM = min(IDX_LIMIT, n_nodes)

    _patch_tile_swdge_sems()
    nc = bacc.Bacc("TRN2", target_bir_lowering=False, debug=False,
                   num_swdge_queues=4)

    dram_in = lambda name, shape, dt: nc.dram_tensor(name, shape, dt,
                                                     kind="ExternalInput")
    xT = dram_in("xT", [DIN, npc], F32)
    W0 = dram_in("W0", [DIN, DH], F32)
    W1 = dram_in("W1", [DH, DH], F32)
    Wout = dram_in("Wout", [DH, DOUT], F32)
    b0_col = dram_in("b0_col", [DH, 1], F32)
    b1_col = dram_in("b1_col", [DH, 1], F32)
    ident_in = dram_in("ident", [128, 128], F32)
    att0_rep = dram_in("att0_rep", [128, DH], F32)
    attsk_rep = dram_in("attsk_rep", [128, KHOPS * DH], BF16)
    hatt_h_rep = dram_in("hatt_h_rep", [128, KHOPS * DH], F32)
    hatt_z_rep = dram_in("hatt_z_rep", [128, KHOPS * DH], F32)
    hbias_rep = dram_in("hbias_rep", [128, (KHOPS + 1) * H], F32)
    pstat_in = dram_in("pstat", [128, lay["P"] * BLK], BF16)
    row_idx_in = dram_in("row_idx", [128, max(lay["row_idx_w"], 1)], I16)
    col_idx_in = dram_in("col_idx", [128, max(lay["col_idx_w"], 1)], I16)

    out_ext = nc.dram_tensor("out", [DOUT, npc], F32, kind="ExternalOutput")
    hdbg = nc.dram_tensor("hdbg", [npc, DH], F32, kind="ExternalOutput") if os.environ.get("KERNEL_DEBUG") else None

    npc_pad = nblk * BLK
    h_hbm = nc.dram_tensor("h_hbm", [npc_pad, DH], F32)
    z_hbm = nc.dram_tensor("z_hbm", [npc_pad, DH], F32)
    zs_own = nc.dram_tensor("zs_own", [npc_pad, DH], BF16)
    zs_own_lo = nc.dram_tensor("zs_own_lo", [HALF, DH], BF16)
    zs_own_hi = nc.dram_tensor("zs_own_hi", [HIPAD, DH], BF16)
    ht_own_lo = nc.dram_tensor("ht_own_lo", [HALF, DH], BF16)
    ht_own_hi = nc.dram_tensor("ht_own_hi", [HIPAD, DH], BF16)
    zs_lo = nc.dram_tensor("zs_lo", [NCORES * HALF, DH], BF16,
                           addr_space="Shared")
    zs_hi = nc.dram_tensor("zs_hi", [NCORES * HIW, DH], BF16,
                           addr_space="Shared")
    ht_lo = nc.dram_tensor("ht_lo", [NCORES * HALF, DH], BF16,
                           addr_space="Shared")
    ht_hi = nc.dram_tensor("ht_hi", [NCORES * HIW, DH], BF16,
                           addr_space="Shared")

    with tile.TileContext(nc) as tc:
        with (
            tc.tile_pool(name="const", bufs=1) as constp,
            tc.tile_pool(name="state", bufs=2) as statep,
            tc.tile_pool(name="gath", bufs=2) as gathp,
            tc.tile_pool(name="zcg", bufs=2) as zcp,
            tc.tile_pool(name="work", bufs=2) as workp,
            tc.tile_pool(name="small", bufs=2) as smallp,
            tc.tile_pool(name="hold", bufs=1) as holdp,
            tc.tile_pool(name="psA", bufs=2, space="PSUM") as psA,
            tc.tile_pool(name="psB", bufs=2, space="PSUM") as psB,
        ):
            gp, ve, se, te = nc.gpsimd, nc.vector, nc.scalar, nc.tensor
            sy = nc.sync

            gp.load_library(library_config.mlp)

            # Round-robin SWDGE gathers over all 4 queues in <=2048-idx
            # pieces: two pieces fit in a queue's 4096-descriptor ring, and
            # 4 active queues give ~4x the random-read bandwidth of one
            # (each SDMA engine keeps one outstanding read per queue).
            qctr = [0]

            def emit_gather(dst, tab, idx_sb_, c0, W, slot0):
                """Gather W*16 rows of tab into dst[:, slot0:...]; W cols of
                16 idxs, W % 8 == 0."""
                off = 0
                while off < W:
                    w = min(128, W - off)
                    s0 = slot0 + off // 8
                    gp.dma_gather(dst[:, s0:s0 + w // 8, :], tab,
                                  idx_sb_[:, c0 + off:c0 + off + w],
                                  w * 16, w * 16, DH, single_packet=False,
                                  queue_num=qctr[0] % 4)
                    qctr[0] += 1
                    off += w

            def ctile(shape, dt, tag, src):
                t = constp.tile(shape, dt, tag=tag)
                sy.dma_start(t[:], src[:])
                return t

            w0_sb = constp.tile([128, 2, DH], F32, tag="w0")
            for kc in range(2):
                sy.dma_start(w0_sb[:, kc, :], W0[kc * 128:(kc + 1) * 128, :])
            w1_sb = ctile([DH, DH], F32, "w1", W1)
            wout_sb = ctile([DH, DOUT], F32, "wout", Wout)
            rowi_sb = ctile([128, max(lay["row_idx_w"], 1)], I16, "rowi",
                            row_idx_in)
            coli_sb = ctile([128, max(lay["col_idx_w"], 1)], I16, "coli",
                            col_idx_in)
            b0_sb = ctile([DH, 1], F32, "b0", b0_col)
            b1_sb = ctile([DH, 1], F32, "b1", b1_col)
            ident_sb = ctile([128, 128], F32, "ident", ident_in)
            att0_sb = ctile([128, DH], F32, "att0", att0_rep)
            attsk_sb = ctile([128, KHOPS * DH], BF16, "attsk", attsk_rep)
            hatth_sb = ctile([128, KHOPS * DH], F32, "hatth", hatt_h_rep)
            hattz_sb = ctile([128, KHOPS * DH], F32, "hattz", hatt_z_rep)
            hbias_sb = ctile([128, (KHOPS + 1) * H], F32, "hbias", hbias_rep)

            def elu_(dst, src, pool, tag):
                # elu(x) = (max(x,0) - 1) + min(e^x, 1); inputs are O(1) so
                # the direct Exp cannot overflow. 1 ACT + 2 DVE passes.
                p = src.shape[0]
                rest = list(src.shape[1:])
                mn = pool.tile([128] + rest, F32, tag=tag + "_mn")
                ex = pool.tile([128] + rest, F32, tag=tag + "_ex")
                se.activation(ex[:p], src, AF.Exp)
                ve.tensor_scalar(mn[:p], src, 0.0, -1.0, OP.max, OP.add)
                ve.scalar_tensor_tensor(dst, ex[:p], 1.0, mn[:p], OP.min, OP.add)

            def hc(apv):
                return apv.rearrange("p (h c) -> p h c", c=C)

            def dram_g(dt, b0, gn):
                return (dt[b0 * BLK:(b0 + gn) * BLK, :]
                        .rearrange("(g p) d -> p g d", p=BLK))

            def dram_half(dt_lo, dt_hi, b0, gn):
                if b0 < 25:
                    return dram_g(dt_lo, b0, gn)
                return dram_g(dt_hi, b0 - 25, gn)

            def ag(ins_t, ins_n, outs_t):
                gp.collective_compute(
                    "AllGather", OP.bypass,
                    replica_groups=[list(range(NCORES))],
                    ins=[ins_t[0:ins_n, :]], outs=[outs_t[:]],
                )

            # =========== MLP + k=0 ===========
            for b in range(nblk):
                nb = min(BLK, npc - b * BLK)
                xt_sb = statep.tile([128, 2, BLK], F32, tag="xt")
                for kc in range(2):
                    sy.dma_start(xt_sb[:, kc, :nb],
                                 xT[kc * 128:(kc + 1) * 128, b * BLK:b * BLK + nb])
                ps = psA.tile([128, BLK], F32, tag="mm")
                for kc in range(2):
                    te.matmul(ps[:, :nb], w0_sb[:, kc, :], xt_sb[:, kc, :nb],
                              start=(kc == 0), stop=(kc == 1))
                h1t = statep.tile([128, BLK], F32, tag="h1t")
                ve.tensor_tensor(h1t[:, :nb], ps[:, :nb],
                                 b0_sb[:, 0:1].broadcast_to([DH, nb]), OP.add)
                elu_(h1t[:, :nb], h1t[:, :nb], statep, "melu")
                ps2 = psA.tile([128, BLK], F32, tag="mm")
                te.matmul(ps2[:, :nb], w1_sb[:], h1t[:, :nb], start=True, stop=True)
                h2t = statep.tile([128, BLK], F32, tag="h2t")
                if nb < BLK:
                    ve.memset(h2t[:, :], 0.0)
                ve.tensor_tensor(h2t[:, :nb], ps2[:, :nb],
                                 b1_sb[:, 0:1].broadcast_to([DH, nb]), OP.add)
                ps3 = psA.tile([128, BLK], F32, tag="mm")
                te.matmul(ps3[:, :], h2t[:, :], ident_sb[:, :],
                          is_transpose=True, start=True, stop=True)
                hfin = statep.tile([128, DH], F32, tag="hfin")
                ve.tensor_copy(hfin[:, :], ps3[:, :])
                sy.dma_start(h_hbm[b * BLK:(b + 1) * BLK, :], hfin[:, :])
                if hdbg is not None:
                    sy.dma_start(hdbg[b * BLK:b * BLK + nb, :], hfin[:nb])

            # k=0 gate/update, grouped over node blocks; zs written in lo/hi
            # halves so the first AllGathers launch as each half completes.
            GB = 8
            for gi0, (g0i, gn) in enumerate(G7):

                def grp0(t):
                    return t[:, :gn, :]

                def hcg0(apv):
                    return apv[:, :gn, :].rearrange("p g (h c) -> p g h c", c=C)

                hng = statep.tile([128, GB, DH], F32, tag="ht_h")
                sy.dma_start(grp0(hng), dram_g(h_hbm, g0i, gn))
                eh = statep.tile([128, GB, DH], F32, tag="tl_e1")
                elu_(grp0(eh), grp0(hng), statep, "tl1")
                ve.tensor_tensor(grp0(eh), grp0(eh),
                                 _mid_bcast(att0_sb[:, :], gn, 1), OP.mult)
                g0v = statep.tile([128, GB, H], F32, tag="tl_gv")
                ve.tensor_reduce(g0v[:, :gn, :], hcg0(eh), AX.X, OP.add)
                ve.tensor_tensor(g0v[:, :gn, :], g0v[:, :gn, :],
                                 _mid_bcast(hbias_sb[:, 0:H], gn, 1), OP.add)
                ztg = statep.tile([128, GB, DH], F32, tag="tl_zb")
                ve.tensor_tensor(hcg0(ztg), hcg0(hng),
                                 g0v[:, :gn, :].broadcast_to([128, gn, H, C]),
                                 OP.mult)
                sy.dma_start(dram_g(z_hbm, g0i, gn), grp0(ztg))
                zsg = statep.tile([128, GB, DH], BF16, tag="tl_zso")
                ve.tensor_scalar(grp0(zsg), grp0(ztg), _decay(0), None, OP.mult)
                sy.dma_start(dram_g(zs_own, g0i, gn), grp0(zsg))
                sy.dma_start(dram_half(zs_own_lo, zs_own_hi, g0i, gn),
                             grp0(zsg))
                if gi0 == 3:
                    ag(zs_own_lo, HALF, zs_lo)
            ag(zs_own_hi, HIW, zs_hi)

            # =========== hops ===========
            for k in range(1, KHOPS + 1):
                # zs_lo/zs_hi AllGathers for this hop were already issued at
                # the end of the previous hop's update loop (or after k0).
                a_all = holdp.tile([128, T, H], BF16, tag="a_all")
                deg_lo = holdp.tile([128, 25, H], F32, tag="deg_lo")
                deg_hi = holdp.tile([128, nblk - 25, H], F32, tag="deg_hi")
                dinv_lo = holdp.tile([128, 25, H], F32, tag="dinv_lo")
                dinv_hi = holdp.tile([128, nblk - 25, H], F32, tag="dinv_hi")
                aself_sb = holdp.tile([128, nblk, H], F32, tag="aself")

                # self-loop attention a_self = softplus(att . elu(2 zs)) + eps
                # computed node-locally (self-loops are excluded from the
                # edge gathers/scatters entirely).
                for (g0a, gna) in G7:
                    zsg_t = statep.tile([128, GB, DH], BF16, tag="tl_zso")
                    sy.dma_start(zsg_t[:, :gna, :], dram_g(zs_own, g0a, gna))
                    x2 = statep.tile([128, GB, DH], F32, tag="tl_e1")
                    ve.tensor_scalar(x2[:, :gna, :], zsg_t[:, :gna, :], 2.0,
                                     None, OP.mult)
                    elu_(x2[:, :gna, :], x2[:, :gna, :], statep, "tl1")
                    ve.tensor_tensor(
                        x2[:, :gna, :], x2[:, :gna, :],
                        _mid_bcast(attsk_sb[:, (k - 1) * DH:k * DH], gna, 1),
                        OP.mult)
                    ar = statep.tile([128, GB, H], F32, tag="tl_gv")
                    ve.tensor_reduce(ar[:, :gna, :],
                                     x2[:, :gna, :].rearrange(
                                         "p g (h c) -> p g h c", c=C),
                                     AX.X, OP.add)
                    asl = statep.tile([128, GB, H], F32, tag="tl_gv2")
                    se.activation(asl[:, :gna, :], ar[:, :gna, :], AF.Softplus)
                    ve.tensor_scalar(aself_sb[:, g0a:g0a + gna, :],
                                     asl[:, :gna, :], 1e-6, None, OP.add)

                def dinv_v(b):
                    return dinv_lo[:, b, :] if b < 25 else dinv_hi[:, b - 25, :]

                def emit_dinv_ht(half):
                    if half == 0:
                        ve.reciprocal(dinv_lo[:], deg_lo[:])
                        se.activation(dinv_lo[:], dinv_lo[:], AF.Sqrt)
                        groups = G7[:4]
                    else:
                        ve.reciprocal(dinv_hi[:], deg_hi[:])
                        se.activation(dinv_hi[:], dinv_hi[:], AF.Sqrt)
                        groups = G7[4:]
                    for (g0, gn) in groups:
                        hgrp = statep.tile([128, GB, DH], F32, tag="ht_h")
                        sy.dma_start(hgrp[:, :gn, :], dram_g(h_hbm, g0, gn))
                        htg = statep.tile([128, GB, DH], BF16, tag="ht_o")
                        dv = (dinv_lo[:, g0:g0 + gn, :] if half == 0
                              else dinv_hi[:, g0 - 25:g0 - 25 + gn, :])
                        ve.tensor_tensor(
                            htg[:, :gn, :].rearrange("p g (h c) -> p g h c", c=C),
                            hgrp[:, :gn, :].rearrange("p g (h c) -> p g h c", c=C),
                            dv.broadcast_to([128, gn, H, C]),
                            OP.mult)
                        sy.dma_start(dram_half(ht_own_lo, ht_own_hi, g0, gn),
                                     htg[:, :gn, :])

                # zc gathers don't depend on the AllGather; prefetch them so
                # the queues stay busy while zs_full lands.
                zc_ring = {}

                def issue_zc(cj):
                    if cj >= len(lay["chunks"]):
                        return
                    chj = lay["chunks"][cj]
                    _, _, _, cc0j, cWj = lay["chunk_idx"][cj]
                    zcj = zcp.tile([128, chj["S"], DH], BF16, tag="g_zc")
                    emit_gather(zcj, zs_own[:, :], coli_sb, cc0j, cWj, 0)
                    zc_ring[cj] = zcj

                issue_zc(0)
                for ci, ch in enumerate(lay["chunks"]):
                    t0, S, nlo = ch["t0"], ch["S"], ch["nlo"]
                    rc0, loW, hiW, cc0, cW = lay["chunk_idx"][ci]
                    zr = gathp.tile([128, S, DH], BF16, tag="g_a")
                    if loW:
                        emit_gather(zr, zs_lo[:, :], rowi_sb, rc0, loW, 0)
                    if hiW:
                        emit_gather(zr, zs_hi[:, :], rowi_sb,
                                    rc0 + loW, hiW, nlo)
                    issue_zc(ci + 1)
                    zc = zc_ring.pop(ci)
                    ve.tensor_tensor(zr[:], zr[:], zc[:], OP.add)
                    tmp = workp.tile([128, S, DH], BF16, tag="welu")
                    se.activation(tmp[:], zr[:], AF.Exp)
                    ve.tensor_scalar(zr[:], zr[:], 0.0, -1.0, OP.max, OP.add)
                    ve.scalar_tensor_tensor(zr[:], tmp[:], 1.0, zr[:],
                                            OP.min, OP.add)
                    ve.tensor_tensor(
                        zr[:], zr[:],
                        _mid_bcast(attsk_sb[:, (k - 1) * DH:k * DH], S, 1),
                        OP.mult)
                    araw = smallp.tile([128, S, H], F32, tag="araw")
                    ve.tensor_reduce(
                        araw[:], zr[:].rearrange("p s (h c) -> p s h c", c=C),
                        AX.X, OP.add)
                    a_t = smallp.tile([128, S, H], F32, tag="a_t")
                    se.activation(a_t[:], araw[:], AF.Softplus)
                    a_v = a_all[:, t0:t0 + S, :]
                    ve.tensor_scalar(a_v, a_t[:], 1e-6, None, OP.add)
                    P0, Pc = ch["P0"], ch["P"]
                    pst = gathp.tile([128, Pc, BLK], BF16, tag="pst")
                    sy.dma_start(pst[:].rearrange("p s w -> p (s w)"),
                                 pstat_in[:, P0 * BLK:(P0 + Pc) * BLK])
                    for b in ch["blocks"]:
                        psd = psB.tile([128, H], F32, tag="psd")
                        first = True
                        for (s, p) in ch["mm"][b]:
                            te.matmul(psd[:, :], pst[:, p - P0, :],
                                      a_all[:, t0 + s, :],
                                      start=first, stop=False,
                                      skip_group_check=True)
                            first = False
                        # deg = edge scatter + self-loop contribution
                        if b < 25:
                            ve.tensor_tensor(deg_lo[:, b, :], psd[:, :],
                                             aself_sb[:, b, :], OP.add)
                        else:
                            ve.tensor_tensor(deg_hi[:, b - 25, :], psd[:, :],
                                             aself_sb[:, b, :], OP.add)
                    if ci == 12:
                        # deg for all lo blocks (0-24) is complete: compute
                        # dinv/ht for the lo half and AllGather it while the
                        # hi chunks of pass 1 are still running.
                        emit_dinv_ht(0)
                    if ci == 14:
                        ag(ht_own_lo, HALF, ht_lo)

                emit_dinv_ht(1)
                ag(ht_own_hi, HIW, ht_hi)

                def emit_upd(k_, g0, gn):
                    def grp(t):
                        return t[:, :gn, :]

                    def hcg(apv):
                        return apv[:, :gn, :].rearrange(
                            "p g (h c) -> p g h c", c=C)

                    hng = statep.tile([128, GB, DH], F32, tag="ht_h")
                    sy.dma_start(grp(hng), dram_g(h_hbm, g0, gn))
                    e1 = statep.tile([128, GB, DH], F32, tag="tl_e1")
                    elu_(grp(e1), grp(hng), statep, "tl1")
                    ve.tensor_tensor(
                        grp(e1), grp(e1),
                        _mid_bcast(hatth_sb[:, (k_ - 1) * DH:k_ * DH], gn, 1),
                        OP.mult)
                    gv = statep.tile([128, GB, H], F32, tag="tl_gv")
                    ve.tensor_reduce(gv[:, :gn, :], hcg(e1), AX.X, OP.add)
                    zsog = statep.tile([128, GB, DH], BF16, tag="tl_zso")
                    sy.dma_start(grp(zsog), dram_g(zs_own, g0, gn))
                    e2 = statep.tile([128, GB, DH], F32, tag="tl_e1")
                    elu_(grp(e2), grp(zsog), statep, "tl1")
                    ve.tensor_tensor(
                        grp(e2), grp(e2),
                        _mid_bcast(hattz_sb[:, (k_ - 1) * DH:k_ * DH], gn, 1),
                        OP.mult)
                    gv2 = statep.tile([128, GB, H], F32, tag="tl_gv2")
                    ve.tensor_reduce(gv2[:, :gn, :], hcg(e2), AX.X, OP.add)
                    ve.tensor_tensor(gv[:, :gn, :], gv[:, :gn, :],
                                     gv2[:, :gn, :], OP.add)
                    ve.tensor_tensor(
                        gv[:, :gn, :], gv[:, :gn, :],
                        _mid_bcast(hbias_sb[:, k_ * H:(k_ + 1) * H], gn, 1),
                        OP.add)
                    zbg = statep.tile([128, GB, DH], F32, tag="tl_zb")
                    sy.dma_start(grp(zbg), dram_g(z_hbm, g0, gn))
                    hgx = statep.tile([128, GB, DH], F32, tag="tl_e1")
                    ve.tensor_tensor(hcg(hgx), hcg(hng),
                                     gv[:, :gn, :].broadcast_to([128, gn, H, C]),
                                     OP.mult)
                    ve.tensor_tensor(grp(zbg), grp(zbg), grp(hgx), OP.add)
                    sy.dma_start(dram_g(z_hbm, g0, gn), grp(zbg))
                    zsg = statep.tile([128, GB, DH], BF16, tag="tl_zso")
                    ve.tensor_scalar(grp(zsg), grp(zbg), _decay(k_), None,
                                     OP.mult)
                    sy.dma_start(dram_g(zs_own, g0, gn), grp(zsg))
                    sy.dma_start(dram_half(zs_own_lo, zs_own_hi, g0, gn),
                                 grp(zsg))

                UPD_AFTER = {3: 0, 7: 1, 11: 2, 12: 3, 16: 4, 20: 5, 24: 6}
                for ci, ch in enumerate(lay["chunks"]):
                    t0, S, nlo = ch["t0"], ch["S"], ch["nlo"]
                    rc0, loW, hiW, cc0, cW = lay["chunk_idx"][ci]
                    hr = gathp.tile([128, S, DH], BF16, tag="g_a")
                    if loW:
                        emit_gather(hr, ht_lo[:, :], rowi_sb, rc0, loW, 0)
                    if hiW:
                        emit_gather(hr, ht_hi[:, :], rowi_sb,
                                    rc0 + loW, hiW, nlo)
                    m_t = hr
                    ve.tensor_tensor(m_t[:].rearrange("p s (h c) -> p s h c", c=C),
                                     hr[:].rearrange("p s (h c) -> p s h c", c=C),
                                     a_all[:, t0:t0 + S, :]
                                     .broadcast_to([128, S, H, C]), OP.mult)
                    P0, Pc = ch["P0"], ch["P"]
                    pst = gathp.tile([128, Pc, BLK], BF16, tag="pst")
                    sy.dma_start(pst[:].rearrange("p s w -> p (s w)"),
                                 pstat_in[:, P0 * BLK:(P0 + Pc) * BLK])
                    for b in ch["blocks"]:
                        psh = psA.tile([128, DH], F32, tag="psh")
                        first = True
                        for (s, p) in ch["mm"][b]:
                            te.matmul(psh[:, :], pst[:, p - P0, :],
                                      m_t[:, s, :],
                                      start=first, stop=False,
                                      skip_group_check=True)
                            first = False
                        # h_new = dinv * (edge scatter + a_self * ht_own)
                        htb = statep.tile([128, DH], BF16, tag="htb")
                        sy.dma_start(htb[:], dram_half(ht_own_lo, ht_own_hi,
                                                       b, 1)[:, 0, :])
                        sm = statep.tile([128, DH], F32, tag="sm")
                        ve.tensor_tensor(hc(sm[:, :]), hc(htb[:, :]),
                                         aself_sb[:, b, :]
                                         .broadcast_to([128, H, C]), OP.mult)
                        ve.tensor_tensor(sm[:, :], sm[:, :], psh[:, :], OP.add)
                        hn = statep.tile([128, DH], F32, tag="hn")
                        ve.tensor_tensor(hc(hn[:, :]), hc(sm[:, :]),
                                         dinv_v(b).broadcast_to([128, H, C]),
                                         OP.mult)
                        sy.dma_start(h_hbm[b * BLK:(b + 1) * BLK, :], hn[:, :])

                    # z/g update for node groups whose h_new is now complete;
                    # once all lo groups are updated the next hop's zs_lo
                    # AllGather can launch under the rest of pass 2.
                    gi = UPD_AFTER.get(ci)
                    if gi is not None:
                        emit_upd(k, *G7[gi])
                    if ci == 14 and k < KHOPS:
                        ag(zs_own_lo, HALF, zs_lo)
                if k < KHOPS:
                    ag(zs_own_hi, HIW, zs_hi)

            # =========== classifier ===========
            for b in range(nblk):
                nb = min(BLK, npc - b * BLK)
                zb = statep.tile([128, DH], F32, tag="cl_z")
                if nb < BLK:
                    ve.memset(zb[:, :], 0.0)
                sy.dma_start(zb[:nb], z_hbm[b * BLK:b * BLK + nb, :])
                elu_(zb[:], zb[:], statep, "clelu")
                pst_ = psA.tile([128, BLK], F32, tag="mm")
                te.matmul(pst_[:, :], zb[:, :], ident_sb[:, :],
                          is_transpose=True, start=True, stop=True)
                zt = statep.tile([128, BLK], F32, tag="cl_zt")
                ve.tensor_copy(zt[:, :], pst_[:, :])
                pso = psA.tile([DOUT, BLK], F32, tag="mm")
                te.matmul(pso[:, :nb], wout_sb[:], zt[:, :nb], start=True,
                          stop=True)
                ob = statep.tile([DOUT, BLK], F32, tag="cl_o")
                ve.tensor_copy(ob[:, :nb], pso[:, :nb])
                sy.dma_start(out_ext[:, b * BLK:b * BLK + nb], ob[:, :nb])

    nc.compile()
    return nc


# ---------------------------------------------------------------------------
# Entry point
# ---------------------------------------------------------------------------

def kernel(**inputs):
    x = np.asarray(inputs["x"], np.float32)
    edge_index = np.asarray(inputs["edge_index"])
    npc = N // NCORES

    cores = _preprocess(edge_index, npc, N)
    lay, per_core = _unify(cores)
    nc = _build(lay, npc, N)

    hop_att0 = np.asarray(inputs["hop_att0"], np.float32)
    hop_atts = np.asarray(inputs["hop_atts"], np.float32)
    atts = np.asarray(inputs["atts"], np.float32)
    hop_biases = np.asarray(inputs["hop_biases"], np.float32)

    rep = lambda v: np.tile(np.asarray(v, np.float32).reshape(1, -1), (128, 1))
    att0_rep = rep(hop_att0)
    attsk_rep = rep(atts[:KHOPS].reshape(KHOPS, DH))
    hatt_h_rep = rep(hop_atts[:KHOPS, :, :C].reshape(KHOPS, DH))
    hatt_z_rep = rep(hop_atts[:KHOPS, :, C:].reshape(KHOPS, DH))
    hbias_rep = rep(hop_biases[:KHOPS + 1])
    b0_col = np.asarray(inputs["b0"], np.float32).reshape(DH, 1)
    b1_col = np.asarray(inputs["b1"], np.float32).reshape(DH, 1)
    ident = np.eye(128, dtype=np.float32)

    in_maps = []
    for r, u in enumerate(per_core):
        in_maps.append({
            "xT": np.ascontiguousarray(x[r * npc:(r + 1) * npc].T),
            "W0": np.asarray(inputs["W0"], np.float32),
            "W1": np.asarray(inputs["W1"], np.float32),
            "Wout": np.asarray(inputs["Wout"], np.float32),
            "b0_col": b0_col, "b1_col": b1_col, "ident": ident,
            "att0_rep": att0_rep, "attsk_rep": attsk_rep.astype(ml_dtypes.bfloat16),
            "hatt_h_rep": hatt_h_rep, "hatt_z_rep": hatt_z_rep,
            "hbias_rep": hbias_rep,
            "pstat": u["pstat"],
            "row_idx": (u["row_idx"] if u["row_idx"].shape[1]
                        else np.zeros((128, 1), np.int16)),
            "col_idx": (u["col_idx"] if u["col_idx"].shape[1]
                        else np.zeros((128, 1), np.int16)),
        })

    if os.environ.get("KERNEL_SIM"):
        import concourse.bass_interp as bass_interp
        sim = bass_interp.MultiCoreSim(nc, NCORES, ignore_data_errors=True)
        for r in range(NCORES):
            for k, v in in_maps[r].items():
                sim.cores[r].tensor(k)[:] = v
        sim.simulate()
        global _LAST_SIM
        _LAST_SIM = sim
        outs = [np.array(sim.cores[r].mem_tensor("out")) for r in range(NCORES)]
    else:
        from concourse.bass_utils import run_bass_kernel_spmd
        res = run_bass_kernel_spmd(nc, in_maps, list(range(NCORES)),
                                   trace=bool(os.environ.get("KERNEL_TRACE")))
        if os.environ.get("KERNEL_TRACE") and res.exec_time_ns:
            print(f"HW exec time: {res.exec_time_ns} ns")
        outs = [res.results[r]["out"] for r in range(NCORES)]

    out = np.concatenate([o.T for o in outs], axis=0)  # [N, DOUT]
    out = out + np.asarray(inputs["bout"], np.float32)[None, :]
    return out.astype(np.float32)

